# revision 1
# baseline (speedup 1.0000x reference)
"""Self-contained Trainium2 Bass kernel for nn_LunarCausalAttention.

Sharding: 8 cores = 2 batches x 4 head-blocks (4 heads each). Params sliced
per core host-side; per-core partial outputs (over head-blocks) summed on
host during the gather (plus bo).

All matmuls use PE row base 0 (mixing row bases within one PSUM bank is a
fatal HW hazard). Odd-head operand halves are DMA-shifted to partitions
0-63; kp is folded into an on-device effective weight W_eff = Wpc^T @ pq^T.
"""

import math

import ml_dtypes
import numpy as np

import concourse.bacc as bacc
import concourse.bass as bass
import concourse.mybir as mybir
import concourse.tile as tile

EMBED = 1024
D = 64
PLEN = 32
NTOK = 2048
BSZ = 2
SCALING = D ** -0.5
BETA = math.log(2.0)

NH = 4           # heads per core
C = 128          # chunk (token tile)
NCH = NTOK // C  # 16 chunks
F32 = mybir.dt.float32
BF16 = mybir.dt.bfloat16
AX = mybir.AxisListType
AF = mybir.ActivationFunctionType


def _bcast(ap_obj, dim_count, at=1):
    """Insert a stride-0 dim of size dim_count into an AP at free position."""
    pat = [list(p) for p in ap_obj.ap]
    pat.insert(at, [0, dim_count])
    return bass.AP(tensor=ap_obj.tensor, offset=ap_obj.offset, ap=pat)


def build_nc(stage=99):
    nc = bacc.Bacc("TRN2", target_bir_lowering=False, debug=False,
                   num_devices=8)

    xT_d = nc.dram_tensor("xT", [EMBED, NTOK], BF16, kind="ExternalInput")
    pxT_d = nc.dram_tensor("pxT", [EMBED, PLEN], BF16, kind="ExternalInput")
    wqc_d = nc.dram_tensor("wqcT", [EMBED, 4 * C], BF16, kind="ExternalInput")
    bqc_d = nc.dram_tensor("bqc", [4 * C], F32, kind="ExternalInput")
    wpq_d = nc.dram_tensor("wpqT", [EMBED, 2 * C], BF16, kind="ExternalInput")
    bpq_d = nc.dram_tensor("bpq", [2 * C], F32, kind="ExternalInput")
    wpc_d = nc.dram_tensor("wpcR", [D, NH, 8, 128], BF16, kind="ExternalInput")
    bpc_d = nc.dram_tensor("bpc0", [D, NH], BF16, kind="ExternalInput")
    wo_d = nc.dram_tensor("woT", [NH * D, EMBED], BF16, kind="ExternalInput")
    rlen_d = nc.dram_tensor("rlen", [C, NCH], F32, kind="ExternalInput")
    mask_d = nc.dram_tensor("mask", [C, C], F32, kind="ExternalInput")
    id64p_d = nc.dram_tensor("id64p", [128, 2, D], BF16, kind="ExternalInput")
    id128_d = nc.dram_tensor("id128", [128, 128], BF16, kind="ExternalInput")
    out_d = nc.dram_tensor("out", [NTOK, EMBED], F32, kind="ExternalOutput")

    with tile.TileContext(nc) as tc:
        with (
            tc.tile_pool(name="big", bufs=1) as big,
            tc.tile_pool(name="wstr", bufs=4) as wstr,
            tc.tile_pool(name="work", bufs=2) as work,
            tc.tile_pool(name="outp", bufs=2) as outp,
            tc.tile_pool(name="psp", bufs=1, space="PSUM") as psp,
        ):
            # ---- persistent loads ----
            xT = big.tile([128, 8, NTOK], BF16)
            nc.sync.dma_start(out=xT, in_=xT_d.rearrange("(k p) n -> p k n", p=128))
            pxT = big.tile([128, 8, PLEN], BF16)
            nc.sync.dma_start(out=pxT, in_=pxT_d.rearrange("(k p) n -> p k n", p=128))
            bpc0 = big.tile([D, NH], BF16)
            nc.sync.dma_start(out=bpc0, in_=bpc_d.ap())
            wo = big.tile([128, 2, EMBED], BF16)
            nc.sync.dma_start(out=wo, in_=wo_d.rearrange("(k p) o -> p k o", p=128))
            bqc = big.tile([128, 4], F32)
            nc.sync.dma_start(out=bqc, in_=bqc_d.rearrange("(m p) -> p m", p=128))
            bpq = big.tile([128, 2], F32)
            nc.sync.dma_start(out=bpq, in_=bpq_d.rearrange("(m p) -> p m", p=128))
            rlen = big.tile([C, NCH], F32)
            nc.sync.dma_start(out=rlen, in_=rlen_d.ap())
            mask = big.tile([C, C], F32)
            nc.sync.dma_start(out=mask, in_=mask_d.ap())
            id64p = big.tile([128, 2, D], BF16)
            nc.sync.dma_start(out=id64p, in_=id64p_d.ap())
            id128 = big.tile([128, 128], BF16)
            nc.sync.dma_start(out=id128, in_=id128_d.ap())
            ones1 = big.tile([1, 512], BF16)
            nc.vector.memset(ones1, 1.0)

            lin = big.tile([128, 4, NTOK], BF16)      # q(0,1) kv(2,3)
            lin0 = big.tile([D, 4, NTOK], BF16)       # odd halves at base 0
            kvtok = big.tile([128, NCH, NH, D], BF16)
            weff = big.tile([128, 8, NH, PLEN], BF16)
            pq_sb = big.tile([128, 2, PLEN], BF16)
            pq0 = big.tile([D, NH, PLEN], BF16)
            be_sb = big.tile([1, NH * PLEN], BF16)
            S1 = big.tile([D, NH, PLEN], F32)
            S2 = big.tile([128, D], F32)
            S1b = big.tile([D, NH, PLEN], BF16)
            S2b = big.tile([128, D], BF16)

            def q_at0(h, tok):
                g, half = h // 2, h % 2
                return (lin0[:, g, tok] if half else lin[0:D, g, tok])

            def kv_at0(h, tok):
                g, half = h // 2, h % 2
                return (lin0[:, 2 + g, tok] if half else lin[0:D, 2 + g, tok])

            # ---- pq linear (channel-major) ----
            wpq_r = wpq_d.rearrange("(k p) m -> p k m", p=128)
            for m in range(2):
                ps = psp.tile([128, PLEN], F32, tag="pE")
                for k in range(8):
                    wpqt = wstr.tile([128, 128], BF16, tag="wt", name="wpqt")
                    nc.sync.dma_start(out=wpqt,
                                      in_=wpq_r[:, k, m * 128:(m + 1) * 128])
                    nc.tensor.matmul(ps, lhsT=wpqt, rhs=pxT[:, k, :],
                                     start=(k == 0), stop=(k == 7))
                nc.scalar.activation(out=pq_sb[:, m, :], in_=ps, func=AF.Identity,
                                     bias=bpq[:, m:m + 1], scale=1.0)

            # pq0: per-head pq at partitions 0-63 (SBUF->SBUF DMA shift)
            for h in range(NH):
                g, half = h // 2, h % 2
                nc.sync.dma_start(out=pq0[:, h, :],
                                  in_=pq_sb[64 * half:64 * half + D, g, :])

            # bias_eff[h, p] = bpc_h . pq_h[:, p]  (exactness for nonzero bpc)
            be_ps = psp.tile([1, NH, PLEN], F32, tag="pE")
            for h in range(NH):
                nc.tensor.matmul(be_ps[:, h, :], lhsT=bpc0[:, h:h + 1],
                                 rhs=pq0[:, h, :], start=True, stop=True)
            nc.vector.tensor_copy(be_sb, be_ps.rearrange("p h w -> p (h w)"))

            # ---- W_eff[e, (h,p)] = sum_d Wpc[(h,d), e] * pq[h, p, d] ----
            for k in range(8):
                wpck = wstr.tile([D, NH, 128], BF16, tag="wpck", bufs=2)
                nc.sync.dma_start(out=wpck, in_=wpc_d[:, :, k, :])
                ps = psp.tile([128, NH, PLEN], F32, tag="pE")
                for h in range(NH):
                    nc.tensor.matmul(ps[:, h, :], lhsT=wpck[:, h, :],
                                     rhs=pq0[:, h, :], start=True, stop=True)
                nc.scalar.copy(weff[:, k, :, :], ps)

            # ---- q/kv linears (weights streamed; 4 psum banks) ----
            wqc_r = wqc_d.rearrange("(k p) m -> p k m", p=128)
            for m in range(4):
                pss = [psp.tile([128, 512], F32, tag=t, name=f"lin_{t}")
                       for t in ("pA", "pB", "pC", "pD")]
                for k in range(8):
                    wt = wstr.tile([128, 128], BF16, tag="wt")
                    nc.sync.dma_start(out=wt,
                                      in_=wqc_r[:, k, m * 128:(m + 1) * 128])
                    for nt in range(4):
                        nc.tensor.matmul(pss[nt],
                                         lhsT=wt,
                                         rhs=xT[:, k, nt * 512:(nt + 1) * 512],
                                         start=(k == 0), stop=(k == 7))
                for nt in range(4):
                    nc.scalar.activation(
                        out=lin[:, m, nt * 512:(nt + 1) * 512], in_=pss[nt],
                        func=AF.Identity, bias=bqc[:, m:m + 1], scale=1.0)

            # ---- lin0: odd halves shifted to partitions 0-63 ----
            for j in range(4):
                nc.sync.dma_start(out=lin0[:, j, :], in_=lin[D:128, j, :])

            # ---- pattn + softplus pre-phase, channel-major [ (h,p), tok ] ----
            z_cm = big.tile([128, NTOK], BF16)
            for nt in range(4 if stage >= 3 else 0):
                sl = slice(nt * 512, (nt + 1) * 512)
                pps = psp.tile([128, 512], F32, tag="pH", name="pat_ps")
                for k in range(8):
                    nc.tensor.matmul(
                        pps, lhsT=weff[:, k, :, :].rearrange("p h w -> p (h w)"),
                        rhs=xT[:, k, sl], start=(k == 0), stop=False)
                nc.tensor.matmul(pps, lhsT=be_sb, rhs=ones1,
                                 start=False, stop=True)
                nc.scalar.activation(out=z_cm[:, sl], in_=pps, func=AF.Exp,
                                     scale=BETA)
            if stage >= 3:
                nc.scalar.activation(out=z_cm, in_=z_cm, func=AF.Ln, bias=1.0)

            # ---- kv transposes to token-major (padded-identity trick) ----
            for c in range(NCH if stage >= 2 else 0):
                tok = slice(c * C, (c + 1) * C)
                ps = psp.tile([128, NH, D], F32, tag="pH")
                for h in range(NH):
                    g, half = h // 2, h % 2
                    nc.tensor.matmul(ps[:, h, :], lhsT=lin[:, 2 + g, tok],
                                     rhs=id64p[:, half, :],
                                     start=True, stop=True)
                nc.vector.tensor_copy(kvtok[:, c], ps)

            # ---- chunk scan loop ----
            for c in range(NCH if stage >= 3 else 0):
                tok = slice(c * C, (c + 1) * C)
                # z token-major for this chunk (single PE transpose)
                ztp = psp.tile([128, 128], BF16, tag="pF", name="ztp")
                nc.tensor.matmul(ztp, lhsT=z_cm[:, tok], rhs=id128,
                                 start=True, stop=True, is_transpose=True)
                z_sb = work.tile([128, 128], BF16, tag="z_sb")
                nc.vector.tensor_copy(z_sb, ztp)

                pd = psp.tile([128, 384], F32, tag="pD")
                # M1 + mask (all heads at row base 0)
                m1 = psp.tile([128, NH, C], F32, tag="pA")
                for h in range(NH):
                    nc.tensor.matmul(m1[:, h, :], lhsT=kv_at0(h, tok),
                                     rhs=q_at0(h, tok), start=True, stop=True)
                m1m = work.tile([128, NH, C], BF16, tag="m1m")
                nc.vector.tensor_mul(m1m, m1, _bcast(mask, NH))

                # out1 = intra + inter  -> pd[:, 0:128]
                for h in range(NH):
                    o1 = pd[:, h * PLEN:(h + 1) * PLEN]
                    nc.tensor.matmul(o1, lhsT=m1m[:, h, :],
                                     rhs=z_sb[:, h * PLEN:(h + 1) * PLEN],
                                     start=True, stop=(c == 0))
                    if c > 0:
                        nc.tensor.matmul(o1, lhsT=q_at0(h, tok),
                                         rhs=S1b[:, h, :],
                                         start=False, stop=True)

                # dS1 / dS2 (pG) + state updates (in place)
                pg = psp.tile([128, 192], F32, tag="pG")
                dS1 = pg[0:D, 64:192].rearrange("p (h w) -> p h w", w=PLEN)
                for h in range(NH):
                    nc.tensor.matmul(dS1[:, h, :], lhsT=kvtok[:, c, h, :],
                                     rhs=z_sb[:, h * PLEN:(h + 1) * PLEN],
                                     start=True, stop=True)
                    nc.tensor.matmul(pg[32 * h:32 * h + 32, 0:D],
                                     lhsT=z_sb[:, h * PLEN:(h + 1) * PLEN],
                                     rhs=kvtok[:, c, h, :],
                                     start=True, stop=True,
                                     tile_position=(0, 32 * h))
                def update_states():
                    if c == 0:
                        nc.vector.tensor_copy(S1, dS1)
                        nc.vector.tensor_copy(S2, pg[:, 0:D])
                    else:
                        nc.vector.tensor_add(S1, dS1, S1)
                        nc.vector.tensor_add(S2, pg[:, 0:D], S2)
                    nc.vector.tensor_copy(S1b, S1)
                    nc.vector.tensor_copy(S2b, S2)

                # softmax over plen (batched across heads) + rlen scales
                t0 = work.tile([128, NH * PLEN], F32, tag="t0")
                nc.vector.tensor_scalar_mul(t0, pd[:, 0:128], rlen[:, c:c + 1])
                nmx = work.tile([128, 1], F32, tag="nmx")
                nc.vector.reduce_max(nmx, t0, axis=AX.X, negate=True)
                e_sb = work.tile([128, NH, PLEN], F32, tag="e_sb")
                nc.scalar.activation(out=e_sb,
                                     in_=t0.rearrange("p (h w) -> p h w", h=NH),
                                     func=AF.Exp, bias=nmx, scale=1.0)
                ssum = work.tile([128, NH], F32, tag="ssum")
                nc.vector.reduce_sum(ssum, e_sb, axis=AX.X)
                rs = work.tile([128, NH], F32, tag="rs")
                nc.vector.reciprocal(rs, ssum)
                rs2 = work.tile([128, NH], F32, tag="rs2")
                nc.vector.tensor_scalar_mul(rs2, rs, rlen[:, c:c + 1])
                aw = work.tile([128, NH, PLEN], BF16, tag="aw")
                nc.vector.tensor_mul(aw, e_sb, _bcast(rs2, PLEN, at=2))

                if stage < 5:
                    update_states()
                    continue
                # aw transpose -> awT stacked [(h,p), tok] (rows 32h per head)
                awp = psp.tile([128, 128], BF16, tag="pF", name="awp")
                nc.tensor.matmul(awp, lhsT=aw.rearrange("p h w -> p (h w)"),
                                 rhs=id128, start=True, stop=True,
                                 is_transpose=True)
                awT = work.tile([128, 128], BF16, tag="awT")
                nc.vector.tensor_copy(awT, awp)

                # M2 + mask: row base 32h, alternating psum banks (pB/pE)
                m2m = []
                for h in range(NH):
                    p0 = 32 * h
                    m2h = psp.tile([128, 128], F32, tag=("pB" if h % 2 == 0
                                                         else "pE"),
                                   name=f"m2h{h % 2}")
                    nc.tensor.matmul(m2h, lhsT=z_cm[p0:p0 + 32, tok],
                                     rhs=awT[p0:p0 + 32, :],
                                     start=True, stop=True,
                                     tile_position=(p0, 0))
                    mm = work.tile([128, 128], BF16, tag=f"m2m{h % 2}")
                    nc.vector.tensor_mul(mm, m2h, mask)
                    m2m.append(mm)

                if stage < 6:
                    update_states()
                    continue
                # out2T intra -> pd[:, 128:384]
                for h in range(NH):
                    g, half = h // 2, h % 2
                    nc.tensor.matmul(
                        pd[64 * half:64 * half + 64,
                           128 + g * C:128 + (g + 1) * C],
                        lhsT=kvtok[:, c, h, :], rhs=m2m[h],
                        start=True, stop=True, tile_position=(0, 64 * half))
                attnT = work.tile([128, 2, C], BF16, tag="attnT")
                nc.scalar.copy(attnT,
                               pd[:, 128:384].rearrange("p (g w) -> p g w", w=C))
                # out2T inter: row base 32h, alternating banks; add into attnT
                if c > 0:
                    for h in range(NH):
                        g, half = h // 2, h % 2
                        p0 = 32 * h
                        o2h = psp.tile([128, 128], F32,
                                       tag=("pB" if h % 2 == 0 else "pE"),
                                       name=f"o2h{h % 2}")
                        nc.tensor.matmul(o2h[64 * half:64 * half + 64, :],
                                         lhsT=S2b[p0:p0 + 32, :],
                                         rhs=awT[p0:p0 + 32, :],
                                         start=True, stop=True,
                                         tile_position=(p0, 64 * half))
                        nc.vector.tensor_add(
                            attnT[64 * half:64 * half + 64, g, :],
                            o2h[64 * half:64 * half + 64, :],
                            attnT[64 * half:64 * half + 64, g, :])

                update_states()

                # final projection; bo added on host during gather
                for nh in range(2):
                    osl = slice(nh * 512, (nh + 1) * 512)
                    fp = psp.tile([128, 512], F32, tag="pC")
                    for kt in range(2):
                        nc.tensor.matmul(fp, lhsT=attnT[:, kt, :],
                                         rhs=wo[:, kt, osl],
                                         start=(kt == 0), stop=(kt == 1))
                    ob = outp.tile([128, 512], F32, tag="ob")
                    if nh == 0:
                        nc.vector.tensor_copy(ob, fp)
                    else:
                        nc.scalar.copy(ob, fp)
                    nc.sync.dma_start(out=out_d[tok, osl], in_=ob)

    nc.compile()
    return nc


_NC = None


def get_nc():
    global _NC
    if _NC is None:
        _NC = build_nc()
    return _NC


def make_in_maps(query, pquery, Wpq, bpq, Wq, bq, Wpc, bpc, Wc, bc, Wo, bo):
    query = np.asarray(query, np.float32)
    pquery = np.asarray(pquery, np.float32)
    Wpq, Wq, Wpc, Wc, Wo = (np.asarray(w, np.float32)
                            for w in (Wpq, Wq, Wpc, Wc, Wo))
    bpq_, bq_, bpc_, bc_ = (np.asarray(v, np.float32)
                            for v in (bpq, bq, bpc, bc))
    n_idx = np.arange(NTOK, dtype=np.float64)
    rlen = (1.0 / ((n_idx + 1.0) * BETA)).astype(np.float32)
    rlen = np.ascontiguousarray(rlen.reshape(NCH, C).T)          # [C, NCH]
    mask = np.triu(np.ones((C, C), np.float32))                  # keep j <= i
    id64p = np.zeros((128, 2, D), np.float32)
    id64p[np.arange(64), 0, np.arange(64)] = 1.0
    id64p[np.arange(64, 128), 1, np.arange(64)] = 1.0
    id128 = np.eye(128, dtype=np.float32)

    in_maps = []
    for core in range(8):
        b, hb = core // 4, core % 4
        ch = slice(hb * NH * D, (hb + 1) * NH * D)
        wqcT = np.concatenate([SCALING * Wq[ch], Wc[ch]], axis=0).T
        bqc = np.concatenate([SCALING * bq_[ch], bc_[ch]])
        wpcR = np.ascontiguousarray(
            Wpc[ch].reshape(NH, D, 8, 128).transpose(1, 0, 2, 3))
        bf = ml_dtypes.bfloat16
        in_maps.append({
            "xT": np.ascontiguousarray(query[:, b, :].T).astype(bf),
            "pxT": np.ascontiguousarray(pquery[:, b, :].T).astype(bf),
            "wqcT": np.ascontiguousarray(wqcT).astype(bf),
            "bqc": np.ascontiguousarray(bqc),
            "wpqT": np.ascontiguousarray((SCALING * Wpq[ch]).T).astype(bf),
            "bpq": np.ascontiguousarray(SCALING * bpq_[ch]),
            "wpcR": wpcR.astype(bf),
            "bpc0": np.ascontiguousarray(bpc_[ch].reshape(NH, D).T).astype(bf),
            "woT": np.ascontiguousarray(Wo[:, ch].T).astype(bf),
            "rlen": rlen, "mask": mask,
            "id64p": id64p.astype(bf), "id128": id128.astype(bf),
        })
    return in_maps


def kernel(**inputs):
    from concourse.bass_utils import run_bass_kernel_spmd
    nc = get_nc()
    in_maps = make_in_maps(**inputs)
    res = run_bass_kernel_spmd(nc, in_maps, core_ids=list(range(8)))
    bo = np.asarray(inputs["bo"], np.float32)
    out = np.zeros((NTOK, BSZ, EMBED), np.float32)
    for b in range(BSZ):
        acc = res.results[4 * b]["out"].astype(np.float32).copy()
        for i in range(1, 4):
            acc += res.results[4 * b + i]["out"]
        out[:, b, :] = acc + bo
    return out



# revision 7
# speedup vs baseline: 1.0010x; 1.0010x over previous
"""Self-contained Trainium2 Bass kernel for nn_LunarCausalAttention (v2).

Sharding: 8 cores = 2 batches x 4 head-blocks (4 heads each). Params sliced
per core host-side; per-core partial outputs (over head-blocks) summed on
host during the gather (plus bo). Output is bf16 on device, f32 on host.

v2 restructure vs v1: parallel-prefix chunk scan (phase C computes all
per-chunk state deltas independently; a small serial DVE prefix produces
per-chunk states; phase E consumes them chunk-independently), block-diagonal
padded S2 so inter-chunk terms chain into the intra PSUM accumulation group,
out1 computed transposed (cheap 32-col LDWEIGHTS) + one PE transpose,
single-DMA weight loads, bf16 output DMA.

PSUM tags (8 banks): tA lin/M1, tD lin/weff/attn, tE pattn/M2,
tT phaseC transposes, tS dS, tU pq/beT/o1t/o1/awp, tF proj (2 banks).
"""

import math

import ml_dtypes
import numpy as np

import concourse.bacc as bacc
import concourse.bass as bass
import concourse.mybir as mybir
import concourse.tile as tile

EMBED = 1024
D = 64
PLEN = 32
NTOK = 2048
BSZ = 2
SCALING = D ** -0.5
BETA = math.log(2.0)

NH = 4           # heads per core
C = 128          # chunk (token tile)
NCH = NTOK // C  # 16 chunks
F32 = mybir.dt.float32
BF16 = mybir.dt.bfloat16
AX = mybir.AxisListType
AF = mybir.ActivationFunctionType

# smalls_f32 column layout
SF_BQC = 0          # [128, 4]
SF_BPQ = 4          # [128, 2]
SF_RLEN = 6         # [128, 16]
SF_MASK = 22        # [128, 128]
SF_ID128F = 150     # [128, 128] f32 identity
SF_COLS = 278
# smalls_bf16 column layout
SB_ID128 = 0        # [128, 128] bf16 identity
SB_BPC = 128        # [64, 4] bpc (heads cols)
SB_COLS = 132


def _bcast(ap_obj, dim_count, at=1):
    """Insert a stride-0 dim of size dim_count into an AP at free position."""
    pat = [list(p) for p in ap_obj.ap]
    pat.insert(at, [0, dim_count])
    return bass.AP(tensor=ap_obj.tensor, offset=ap_obj.offset, ap=pat)


def build_nc(stage=6):
    nc = bacc.Bacc("TRN2", target_bir_lowering=False, debug=False,
                   num_devices=8)

    xT_d = nc.dram_tensor("xT", [EMBED, NTOK], BF16, kind="ExternalInput")
    pxT_d = nc.dram_tensor("pxT", [EMBED, PLEN], BF16, kind="ExternalInput")
    wqc_d = nc.dram_tensor("wqcT", [EMBED, 4 * C], BF16, kind="ExternalInput")
    wpq_d = nc.dram_tensor("wpqT", [EMBED, 2 * C], BF16, kind="ExternalInput")
    wpc_d = nc.dram_tensor("wpcR", [D, NH, 8, 128], BF16, kind="ExternalInput")
    wo_d = nc.dram_tensor("woT", [NH * D, EMBED], BF16, kind="ExternalInput")
    sf_d = nc.dram_tensor("smf", [128, SF_COLS], F32, kind="ExternalInput")
    sb_d = nc.dram_tensor("smb", [128, SB_COLS], BF16, kind="ExternalInput")
    out_d = nc.dram_tensor("out", [NTOK, EMBED], BF16, kind="ExternalOutput")

    with tile.TileContext(nc) as tc:
        with (
            tc.tile_pool(name="big", bufs=1) as big,
            tc.tile_pool(name="work", bufs=2) as work,
            tc.tile_pool(name="outp", bufs=2) as outp,
            tc.tile_pool(name="psp", bufs=1, space="PSUM") as psp,
        ):
            # ---- persistent loads (each a single DMA) ----
            smf = big.tile([128, SF_COLS], F32)
            nc.sync.dma_start(out=smf, in_=sf_d.ap())
            smb = big.tile([128, SB_COLS], BF16)
            nc.sync.dma_start(out=smb, in_=sb_d.ap())
            wqc = big.tile([128, 8, 4 * C], BF16)
            nc.sync.dma_start(out=wqc,
                              in_=wqc_d.rearrange("(k p) m -> p k m", p=128))
            wpq = big.tile([128, 8, 2 * C], BF16)
            nc.sync.dma_start(out=wpq,
                              in_=wpq_d.rearrange("(k p) m -> p k m", p=128))
            wpc = big.tile([D, NH, 8, 128], BF16)
            nc.sync.dma_start(out=wpc, in_=wpc_d.ap())
            wo = big.tile([128, 2, EMBED], BF16)
            nc.sync.dma_start(out=wo,
                              in_=wo_d.rearrange("(k p) o -> p k o", p=128))
            pxT = big.tile([128, 8, PLEN], BF16)
            nc.sync.dma_start(out=pxT,
                              in_=pxT_d.rearrange("(k p) n -> p k n", p=128))
            # xT in 4 token-quarters so compute can start early
            xT = big.tile([128, 8, NTOK], BF16)
            xT_r = xT_d.rearrange("(k p) n -> p k n", p=128)
            NQ = NTOK // 4
            for qi in range(4):
                nc.sync.dma_start(out=xT[:, :, qi * NQ:(qi + 1) * NQ],
                                  in_=xT_r[:, :, qi * NQ:(qi + 1) * NQ])

            bqc = smf[:, SF_BQC:SF_BQC + 4]
            bpq = smf[:, SF_BPQ:SF_BPQ + 2]
            rlen = smf[:, SF_RLEN:SF_RLEN + NCH]
            mask = smf[:, SF_MASK:SF_MASK + C]
            id128f = smf[:, SF_ID128F:SF_ID128F + 128]
            id128 = smb[:, SB_ID128:SB_ID128 + 128]
            bpc0 = smb[0:D, SB_BPC:SB_BPC + NH]

            # ---- persistent compute tensors ----
            lin = big.tile([128, 4, NTOK], BF16)      # q(0,1) kv(2,3) chan-major
            lin0 = big.tile([D, 4, NTOK], BF16)       # odd halves at base 0
            z_cm = big.tile([128, NTOK], BF16)        # [(h,p), tok]
            weff = big.tile([128, 8, NH * PLEN], BF16)
            pq_sb = big.tile([128, 2, PLEN], BF16)
            pq0 = big.tile([D, NH, PLEN], BF16)
            beT = big.tile([128, 1], F32)             # beta * bias_eff per part
            z_sb = big.tile([128, NCH, C], BF16)      # [tok, c, (h,p)]
            kvtok = big.tile([128, NCH, 2, C], BF16)  # [tok, c, g, chan]
            dS_sb = big.tile([128, NCH, 192], F32)    # [0:64]=dS2, [64:192]=dS1
            Scum = big.tile([128, 192], F32)
            S1b = big.tile([D, NCH, NH * PLEN], BF16)  # prefix thru c
            S2b = big.tile([128, NCH, NH, D], BF16)    # block-diag padded

            nc.vector.memset(S2b, 0.0)
            nc.vector.memset(dS_sb, 0.0)

            def q_at0(h, tok):
                g, half = h // 2, h % 2
                return (lin0[:, g, tok] if half else lin[0:D, g, tok])

            def kv_at0(h, tok):
                g, half = h // 2, h % 2
                return (lin0[:, 2 + g, tok] if half else lin[0:D, 2 + g, tok])

            # ---- pq linear (channel-major) ----
            for m in range(2):
                ps = psp.tile([128, 512], F32, tag="tU", name="pq_ps")
                for k in range(8):
                    nc.tensor.matmul(ps[:, 0:PLEN],
                                     lhsT=wpq[:, k, m * 128:(m + 1) * 128],
                                     rhs=pxT[:, k, :],
                                     start=(k == 0), stop=(k == 7))
                nc.scalar.activation(out=pq_sb[:, m, :], in_=ps[:, 0:PLEN],
                                     func=AF.Identity, bias=bpq[:, m:m + 1],
                                     scale=1.0)

            # pq0: per-head pq at partitions 0-63 (SBUF->SBUF DMA shift)
            for h in range(NH):
                g, half = h // 2, h % 2
                nc.sync.dma_start(out=pq0[:, h, :],
                                  in_=pq_sb[64 * half:64 * half + D, g, :])

            # beT[(h,p)] = beta * (bpc_h . pq_h[:, p])  (col-packed matmuls)
            beT_ps = psp.tile([128, 512], F32, tag="tU", name="beT_ps")
            for h in range(NH):
                nc.tensor.matmul(beT_ps[32 * h:32 * h + 32, 0:1],
                                 lhsT=pq0[:, h, :], rhs=bpc0[:, h:h + 1],
                                 start=True, stop=True,
                                 tile_position=(0, 32 * h))
            nc.vector.tensor_scalar_mul(beT, beT_ps[:, 0:1], BETA)

            # ---- W_eff[e, (h,p)] = sum_d Wpc[(h,d), e] * pq[h, p, d] ----
            for k in range(8):
                ps = psp.tile([128, 512], F32, tag="tD", name="weff_ps")
                for h in range(NH):
                    nc.tensor.matmul(ps[:, h * PLEN:(h + 1) * PLEN],
                                     lhsT=wpc[:, h, k, :],
                                     rhs=pq0[:, h, :], start=True, stop=True)
                nc.scalar.copy(weff[:, k, :], ps[:, 0:NH * PLEN])

            # ---- q/kv linears + pattn + softplus, per token-quarter ----
            lin_tags = ("tA", "tD")
            for nt in range(4):
                sl = slice(nt * 512, (nt + 1) * 512)
                for m in range(4):
                    ps = psp.tile([128, 512], F32, tag=lin_tags[m % 2],
                                  name="lin_ps")
                    for k in range(8):
                        nc.tensor.matmul(ps,
                                         lhsT=wqc[:, k, m * 128:(m + 1) * 128],
                                         rhs=xT[:, k, sl],
                                         start=(k == 0), stop=(k == 7))
                    nc.scalar.activation(out=lin[:, m, sl], in_=ps,
                                         func=AF.Identity, bias=bqc[:, m:m + 1],
                                         scale=1.0)
                pps = psp.tile([128, 512], F32, tag="tE", name="pat_ps")
                for k in range(8):
                    nc.tensor.matmul(pps, lhsT=weff[:, k, :], rhs=xT[:, k, sl],
                                     start=(k == 0), stop=(k == 7))
                # z = ln(1 + exp(beta*pattn + beta*be)); /beta folded into rlen
                nc.scalar.activation(out=z_cm[:, sl], in_=pps, func=AF.Exp,
                                     bias=beT[:, 0:1], scale=BETA)
                nc.scalar.activation(out=z_cm[:, sl], in_=z_cm[:, sl],
                                     func=AF.Ln, bias=1.0)

            # odd halves of q/kv shifted to partitions 0-63 (one DMA)
            nc.sync.dma_start(out=lin0, in_=lin[D:128, :, :])

            # ---- phase C: per-chunk transposes + state deltas ----
            for c in range(NCH if stage >= 2 else 0):
                tok = slice(c * C, (c + 1) * C)
                tp = psp.tile([128, 3, C], BF16, tag="tT", name="tp")
                nc.tensor.matmul(tp[:, 0, :], lhsT=z_cm[:, tok], rhs=id128,
                                 start=True, stop=True, is_transpose=True)
                for g in range(2):
                    nc.tensor.matmul(tp[:, 1 + g, :], lhsT=lin[:, 2 + g, tok],
                                     rhs=id128, start=True, stop=True,
                                     is_transpose=True)
                nc.scalar.copy(z_sb[:, c, :], tp[:, 0, :])
                nc.scalar.copy(kvtok[:, c, :, :], tp[:, 1:3, :])

                dsp = psp.tile([128, 192], F32, tag="tS", name="dsp")
                for h in range(NH):
                    g, half = h // 2, h % 2
                    kvs = kvtok[:, c, g, 64 * half:64 * half + D]
                    zs = z_sb[:, c, h * PLEN:(h + 1) * PLEN]
                    # dS1[d, (h,p)]
                    nc.tensor.matmul(
                        dsp[0:D, 64 + h * PLEN:64 + (h + 1) * PLEN],
                        lhsT=kvs, rhs=zs, start=True, stop=True)
                    # dS2[(h,p), d] (col-packed)
                    nc.tensor.matmul(dsp[32 * h:32 * h + 32, 0:D],
                                     lhsT=zs, rhs=kvs, start=True, stop=True,
                                     tile_position=(0, 32 * h))
                nc.vector.tensor_copy(dS_sb[:, c, 0:D], dsp[:, 0:D])
                nc.vector.tensor_copy(dS_sb[0:D, c, 64:192], dsp[0:D, 64:192])

            # ---- phase D: prefix sums (serial DVE chain, small) ----
            for c in range(NCH if stage >= 2 else 0):
                if c == 0:
                    nc.vector.tensor_copy(Scum, dS_sb[:, 0, :])
                else:
                    nc.vector.tensor_add(Scum, dS_sb[:, c, :], Scum)
                # S1b[c] : [d, (h,p)] bf16  (prefix THROUGH c)
                nc.vector.tensor_copy(S1b[:, c, :], Scum[0:D, 64:192])
                # S2b[c] : block-diagonal [(h,p), h, d]
                for h in range(NH):
                    nc.vector.tensor_copy(
                        S2b[32 * h:32 * h + 32, c, h, :],
                        Scum[32 * h:32 * h + 32, 0:D])

            # ---- phase E: per-chunk attention + output ----
            for c in range(NCH if stage >= 3 else 0):
                tok = slice(c * C, (c + 1) * C)
                # M1[key, query] per head
                m1 = psp.tile([128, NH, C], F32, tag="tA", name="m1")
                for h in range(NH):
                    nc.tensor.matmul(m1[:, h, :], lhsT=kv_at0(h, tok),
                                     rhs=q_at0(h, tok), start=True, stop=True)
                m1m = work.tile([128, NH, C], BF16, tag="m1m")
                nc.vector.tensor_mul(m1m, m1, _bcast(mask, NH))

                # out1T[(h,p), query] = intra + inter (col-packed per head)
                o1t = psp.tile([128, C], F32, tag="tU", name="o1t")
                for h in range(NH):
                    nc.tensor.matmul(o1t[32 * h:32 * h + 32, :],
                                     lhsT=z_sb[:, c, h * PLEN:(h + 1) * PLEN],
                                     rhs=m1m[:, h, :],
                                     start=True, stop=(c == 0),
                                     tile_position=(0, 32 * h))
                    if c > 0:
                        nc.tensor.matmul(
                            o1t[32 * h:32 * h + 32, :],
                            lhsT=S1b[:, c - 1, h * PLEN:(h + 1) * PLEN],
                            rhs=q_at0(h, tok),
                            start=False, stop=True,
                            tile_position=(0, 32 * h))
                if stage < 4:
                    continue
                o1t_sb = work.tile([128, C], F32, tag="o1t_sb")
                nc.scalar.copy(o1t_sb, o1t)
                # transpose back to [query, (h,p)] (f32)
                o1 = psp.tile([128, C], F32, tag="tU", name="o1")
                nc.tensor.matmul(o1, lhsT=o1t_sb, rhs=id128f,
                                 start=True, stop=True, is_transpose=True)

                # softmax over plen (no max subtraction; |x| < 20 verified)
                e_sb = work.tile([128, NH, PLEN], F32, tag="e_sb")
                nc.scalar.activation(
                    out=e_sb, in_=o1.rearrange("p (h w) -> p h w", h=NH),
                    func=AF.Exp, scale=rlen[:, c:c + 1])
                ssum = work.tile([128, NH], F32, tag="ssum")
                nc.vector.reduce_sum(ssum, e_sb, axis=AX.X)
                rs = work.tile([128, NH], F32, tag="rs")
                nc.vector.reciprocal(rs, ssum)
                rs2 = work.tile([128, NH], F32, tag="rs2")
                nc.vector.tensor_scalar_mul(rs2, rs, rlen[:, c:c + 1])
                aw = work.tile([128, NH, PLEN], BF16, tag="aw")
                nc.vector.tensor_mul(aw, e_sb, _bcast(rs2, PLEN, at=2))

                if stage < 5:
                    continue
                # awT[(h,p), query]
                awp = psp.tile([128, C], BF16, tag="tU", name="awp")
                nc.tensor.matmul(awp, lhsT=aw.rearrange("p h w -> p (h w)"),
                                 rhs=id128, start=True, stop=True,
                                 is_transpose=True)
                awT = work.tile([128, C], BF16, tag="awT")
                nc.scalar.copy(awT, awp)

                # M2[key, query] per head (rows 32h). Disjoint row groups run
                # CONCURRENTLY in the PE array, so consecutive heads must hit
                # different PSUM banks; alternate tags (tE/tT) so the tag
                # write-after-read dependency serializes same-bank reuse.
                m2m = work.tile([128, NH, C], BF16, tag="m2m")
                for h in range(NH):
                    p0 = 32 * h
                    m2h = psp.tile([128, C], F32,
                                   tag=("tE" if h % 2 == 0 else "tT"),
                                   name=f"m2h{h % 2}")
                    nc.tensor.matmul(m2h, lhsT=z_cm[p0:p0 + 32, tok],
                                     rhs=awT[p0:p0 + 32, :],
                                     start=True, stop=True,
                                     tile_position=(p0, 0))
                    nc.vector.tensor_mul(m2m[:, h, :], m2h, mask)

                if stage < 6:
                    continue
                # out2 = intra + inter, chained into one PSUM group per head
                attn = psp.tile([128, 2, C], F32, tag="tD", name="attn")
                for h in range(NH):
                    g, half = h // 2, h % 2
                    dst = attn[64 * half:64 * half + D, g, :]
                    nc.tensor.matmul(
                        dst,
                        lhsT=kvtok[:, c, g, 64 * half:64 * half + D],
                        rhs=m2m[:, h, :],
                        start=True, stop=(c == 0),
                        tile_position=(0, 64 * half))
                    if c > 0:
                        nc.tensor.matmul(dst, lhsT=S2b[:, c - 1, h, :],
                                         rhs=awT,
                                         start=False, stop=True,
                                         tile_position=(0, 64 * half))
                attnT = work.tile([128, 2, C], BF16, tag="attnT")
                nc.scalar.copy(attnT, attn)

                # final projection -> bf16 out (bo added on host)
                fp = psp.tile([128, EMBED], F32, tag="tF", name="fp")
                for kt in range(2):
                    for nh in range(2):
                        nc.tensor.matmul(fp[:, nh * 512:(nh + 1) * 512],
                                         lhsT=attnT[:, kt, :],
                                         rhs=wo[:, kt, nh * 512:(nh + 1) * 512],
                                         start=(kt == 0), stop=(kt == 1))
                ob = outp.tile([128, EMBED], BF16, tag="ob")
                nc.scalar.copy(ob, fp)
                nc.sync.dma_start(out=out_d[tok, :], in_=ob)

    nc.compile()
    return nc


_NC = None
_NC_STAGE = None


def get_nc(stage=6):
    global _NC, _NC_STAGE
    if _NC is None or _NC_STAGE != stage:
        _NC = build_nc(stage)
        _NC_STAGE = stage
    return _NC


def make_in_maps(query, pquery, Wpq, bpq, Wq, bq, Wpc, bpc, Wc, bc, Wo, bo):
    query = np.asarray(query, np.float32)
    pquery = np.asarray(pquery, np.float32)
    Wpq, Wq, Wpc, Wc, Wo = (np.asarray(w, np.float32)
                            for w in (Wpq, Wq, Wpc, Wc, Wo))
    bpq_, bq_, bpc_, bc_ = (np.asarray(v, np.float32)
                            for v in (bpq, bq, bpc, bc))
    n_idx = np.arange(NTOK, dtype=np.float64)
    rlen = (1.0 / ((n_idx + 1.0) * BETA)).astype(np.float32)
    rlen = np.ascontiguousarray(rlen.reshape(NCH, C).T)          # [C, NCH]
    mask = np.triu(np.ones((C, C), np.float32))                  # keep j <= i
    id128 = np.eye(128, dtype=np.float32)

    bf = ml_dtypes.bfloat16
    in_maps = []
    for core in range(8):
        b, hb = core // 4, core % 4
        ch = slice(hb * NH * D, (hb + 1) * NH * D)
        wqcT = np.concatenate([SCALING * Wq[ch], Wc[ch]], axis=0).T
        bqc = np.concatenate([SCALING * bq_[ch], bc_[ch]])       # (512,)
        bpqs = SCALING * bpq_[ch]                                # (256,)
        wpcR = np.ascontiguousarray(
            Wpc[ch].reshape(NH, D, 8, 128).transpose(1, 0, 2, 3))

        smf = np.zeros((128, SF_COLS), np.float32)
        smf[:, SF_BQC:SF_BQC + 4] = bqc.reshape(4, 128).T
        smf[:, SF_BPQ:SF_BPQ + 2] = bpqs.reshape(2, 128).T
        smf[:, SF_RLEN:SF_RLEN + NCH] = rlen
        smf[:, SF_MASK:SF_MASK + C] = mask
        smf[:, SF_ID128F:SF_ID128F + 128] = id128

        smb = np.zeros((128, SB_COLS), np.float32)
        smb[:, SB_ID128:SB_ID128 + 128] = id128
        smb[0:D, SB_BPC:SB_BPC + NH] = bpc_[ch].reshape(NH, D).T

        in_maps.append({
            "xT": np.ascontiguousarray(query[:, b, :].T).astype(bf),
            "pxT": np.ascontiguousarray(pquery[:, b, :].T).astype(bf),
            "wqcT": np.ascontiguousarray(wqcT).astype(bf),
            "wpqT": np.ascontiguousarray((SCALING * Wpq[ch]).T).astype(bf),
            "wpcR": wpcR.astype(bf),
            "woT": np.ascontiguousarray(Wo[:, ch].T).astype(bf),
            "smf": smf,
            "smb": smb.astype(bf),
        })
    return in_maps


def kernel(**inputs):
    from concourse.bass_utils import run_bass_kernel_spmd
    nc = get_nc()
    in_maps = make_in_maps(**inputs)
    res = run_bass_kernel_spmd(nc, in_maps, core_ids=list(range(8)))
    bo = np.asarray(inputs["bo"], np.float32)
    out = np.zeros((NTOK, BSZ, EMBED), np.float32)
    for b in range(BSZ):
        acc = res.results[4 * b]["out"].astype(np.float32)
        for i in range(1, 4):
            acc = acc + res.results[4 * b + i]["out"].astype(np.float32)
        out[:, b, :] = acc + bo
    return out


# revision 9
# speedup vs baseline: 1.2214x; 1.2202x over previous
"""Self-contained Trainium2 Bass kernel for nn_LunarCausalAttention (v2).

Sharding: 8 cores = 2 batches x 4 head-blocks (4 heads each). Params sliced
per core host-side; per-core partial outputs (over head-blocks) summed on
host during the gather (plus bo). Output is bf16 on device, f32 on host.

v2 restructure vs v1: parallel-prefix chunk scan (phase C computes all
per-chunk state deltas independently; a small serial DVE prefix produces
per-chunk states; phase E consumes them chunk-independently), block-diagonal
padded S2 so inter-chunk terms chain into the intra PSUM accumulation group,
out1 computed transposed (cheap 32-col LDWEIGHTS) + one PE transpose,
single-DMA weight loads, bf16 output DMA.

PSUM tags (8 banks): tA lin/M1, tD lin/weff/attn, tE pattn/M2,
tT phaseC transposes, tS dS, tU pq/beT/o1t/o1/awp, tF proj (2 banks).
"""

import math

import ml_dtypes
import numpy as np

import concourse.bacc as bacc
import concourse.bass as bass
import concourse.mybir as mybir
import concourse.tile as tile

EMBED = 1024
D = 64
PLEN = 32
NTOK = 2048
BSZ = 2
SCALING = D ** -0.5
BETA = math.log(2.0)

NH = 4           # heads per core
C = 128          # chunk (token tile)
NCH = NTOK // C  # 16 chunks
F32 = mybir.dt.float32
BF16 = mybir.dt.bfloat16
AX = mybir.AxisListType
AF = mybir.ActivationFunctionType

# smalls_f32 column layout
SF_BQC = 0          # [128, 4]
SF_BPQ = 4          # [64, 4] (head-major bpq at partitions 0-63)
SF_RLEN = 8         # [128, 16]
SF_MASK = 24        # [128, 128]
SF_COLS = 152
# smalls_bf16 column layout
SB_ID128 = 0        # [128, 128] bf16 identity
SB_BPC = 128        # [64, 4] bpc (heads cols)
SB_COLS = 132


def _bcast(ap_obj, dim_count, at=1):
    """Insert a stride-0 dim of size dim_count into an AP at free position."""
    pat = [list(p) for p in ap_obj.ap]
    pat.insert(at, [0, dim_count])
    return bass.AP(tensor=ap_obj.tensor, offset=ap_obj.offset, ap=pat)


def build_nc(stage=6):
    nc = bacc.Bacc("TRN2", target_bir_lowering=False, debug=False,
                   num_devices=8)

    xT_d = nc.dram_tensor("xT", [EMBED, NTOK], BF16, kind="ExternalInput")
    pxT_d = nc.dram_tensor("pxT", [EMBED, PLEN], BF16, kind="ExternalInput")
    wqc_d = nc.dram_tensor("wqcT", [EMBED, 4 * C], BF16, kind="ExternalInput")
    wpq_d = nc.dram_tensor("wpqT", [EMBED, 2 * C], BF16, kind="ExternalInput")
    wpc_d = nc.dram_tensor("wpcR", [D, NH, 8, 128], BF16, kind="ExternalInput")
    wo_d = nc.dram_tensor("woT", [NH * D, EMBED], BF16, kind="ExternalInput")
    sf_d = nc.dram_tensor("smf", [128, SF_COLS], F32, kind="ExternalInput")
    sb_d = nc.dram_tensor("smb", [128, SB_COLS], BF16, kind="ExternalInput")
    out_d = nc.dram_tensor("out", [NTOK, EMBED], BF16, kind="ExternalOutput")

    with tile.TileContext(nc) as tc:
        with (
            tc.tile_pool(name="big", bufs=1) as big,
            tc.tile_pool(name="work", bufs=2) as work,
            tc.tile_pool(name="outp", bufs=2) as outp,
            tc.tile_pool(name="psp", bufs=1, space="PSUM") as psp,
        ):
            # ---- persistent loads (each a single DMA) ----
            # order: xT q0 + pq-path weights first (critical path), rest after
            xT = big.tile([128, 8, NTOK], BF16)
            xT_r = xT_d.rearrange("(k p) n -> p k n", p=128)
            NQ = NTOK // 4
            nc.sync.dma_start(out=xT[:, :, 0:NQ], in_=xT_r[:, :, 0:NQ])
            wpq = big.tile([128, 8, 2 * C], BF16)
            nc.sync.dma_start(out=wpq,
                              in_=wpq_d.rearrange("(k p) m -> p k m", p=128))
            pxT = big.tile([128, 8, PLEN], BF16)
            nc.sync.dma_start(out=pxT,
                              in_=pxT_d.rearrange("(k p) n -> p k n", p=128))
            smf = big.tile([128, SF_COLS], F32)
            nc.sync.dma_start(out=smf, in_=sf_d.ap())
            smb = big.tile([128, SB_COLS], BF16)
            nc.sync.dma_start(out=smb, in_=sb_d.ap())
            wqc = big.tile([128, 8, 4 * C], BF16)
            nc.sync.dma_start(out=wqc,
                              in_=wqc_d.rearrange("(k p) m -> p k m", p=128))
            wpc = big.tile([D, NH, 8, 128], BF16)
            nc.sync.dma_start(out=wpc, in_=wpc_d.ap())
            wo = big.tile([128, 2, EMBED], BF16)
            nc.sync.dma_start(out=wo,
                              in_=wo_d.rearrange("(k p) o -> p k o", p=128))
            for qi in range(1, 4):
                nc.sync.dma_start(out=xT[:, :, qi * NQ:(qi + 1) * NQ],
                                  in_=xT_r[:, :, qi * NQ:(qi + 1) * NQ])

            bqc = smf[:, SF_BQC:SF_BQC + 4]
            bpq = smf[0:D, SF_BPQ:SF_BPQ + NH]
            rlen = smf[:, SF_RLEN:SF_RLEN + NCH]
            mask = smf[:, SF_MASK:SF_MASK + C]
            id128 = smb[:, SB_ID128:SB_ID128 + 128]
            bpc0 = smb[0:D, SB_BPC:SB_BPC + NH]

            # ---- persistent compute tensors ----
            lin = big.tile([128, 4, NTOK], BF16)      # q(0,1) kv(2,3) chan-major
            lin0 = big.tile([D, 4, NTOK], BF16)       # odd halves at base 0
            z_cm = big.tile([128, NTOK], BF16)        # [(h,p), tok]
            weff = big.tile([128, 8, NH * PLEN], BF16)
            pq0 = big.tile([D, NH, PLEN], BF16)
            beT = big.tile([128, 1], F32)             # beta * bias_eff per part
            z_sb = big.tile([128, NCH, C], BF16)      # [tok, c, (h,p)]
            kvtok = big.tile([128, NCH, 2, C], BF16)  # [tok, c, g, chan]
            dS_sb = big.tile([128, NCH, 192], F32)    # [0:64]=dS2, [64:192]=dS1
            Scum = big.tile([128, 2, 192], F32)
            S1b = big.tile([D, NCH, NH * PLEN], BF16)  # prefix thru c
            S2b = big.tile([128, NCH, NH, D], BF16)    # block-diag padded

            nc.vector.memset(S2b, 0.0)
            nc.vector.memset(dS_sb, 0.0)

            def q_at0(h, tok):
                g, half = h // 2, h % 2
                return (lin0[:, g, tok] if half else lin[0:D, g, tok])

            def kv_at0(h, tok):
                g, half = h // 2, h % 2
                return (lin0[:, 2 + g, tok] if half else lin[0:D, 2 + g, tok])

            # ---- pq linear, per head directly at partitions 0-63 ----
            pq_ps = psp.tile([D, NH, PLEN], F32, tag="tU", name="pq_ps")
            for h in range(NH):
                for k in range(8):
                    nc.tensor.matmul(pq_ps[:, h, :],
                                     lhsT=wpq[:, k, h * D:(h + 1) * D],
                                     rhs=pxT[:, k, :],
                                     start=(k == 0), stop=(k == 7))
            for h in range(NH):
                nc.scalar.activation(out=pq0[:, h, :], in_=pq_ps[:, h, :],
                                     func=AF.Identity, bias=bpq[:, h:h + 1],
                                     scale=1.0)

            # beT[(h,p)] = beta * (bpc_h . pq_h[:, p])  (col-packed matmuls)
            beT_ps = psp.tile([128, 512], F32, tag="tS", name="beT_ps")
            for h in range(NH):
                nc.tensor.matmul(beT_ps[32 * h:32 * h + 32, 0:1],
                                 lhsT=pq0[:, h, :], rhs=bpc0[:, h:h + 1],
                                 start=True, stop=True,
                                 tile_position=(0, 32 * h))
            nc.vector.tensor_scalar_mul(beT, beT_ps[:, 0:1], BETA)

            # ---- W_eff[e, (h,p)] = sum_d Wpc[(h,d), e] * pq[h, p, d] ----
            for k in range(8):
                ps = psp.tile([128, 512], F32, tag="tD", name="weff_ps")
                for h in range(NH):
                    nc.tensor.matmul(ps[:, h * PLEN:(h + 1) * PLEN],
                                     lhsT=wpc[:, h, k, :],
                                     rhs=pq0[:, h, :], start=True, stop=True)
                nc.scalar.copy(weff[:, k, :], ps[:, 0:NH * PLEN])

            # ---- q/kv linears + pattn + softplus, per token-quarter ----
            lin_tags = ("tA", "tD")
            for nt in range(4):
                sl = slice(nt * 512, (nt + 1) * 512)
                for m in range(4):
                    ps = psp.tile([128, 512], F32, tag=lin_tags[m % 2],
                                  name="lin_ps")
                    for k in range(8):
                        nc.tensor.matmul(ps,
                                         lhsT=wqc[:, k, m * 128:(m + 1) * 128],
                                         rhs=xT[:, k, sl],
                                         start=(k == 0), stop=(k == 7))
                    nc.scalar.activation(out=lin[:, m, sl], in_=ps,
                                         func=AF.Identity, bias=bqc[:, m:m + 1],
                                         scale=1.0)
                pps = psp.tile([128, 512], F32, tag="tE", name="pat_ps")
                for k in range(8):
                    nc.tensor.matmul(pps, lhsT=weff[:, k, :], rhs=xT[:, k, sl],
                                     start=(k == 0), stop=(k == 7))
                # z = ln(1 + exp(beta*pattn + beta*be)); /beta folded into rlen
                nc.scalar.activation(out=z_cm[:, sl], in_=pps, func=AF.Exp,
                                     bias=beT[:, 0:1], scale=BETA)
                nc.scalar.activation(out=z_cm[:, sl], in_=z_cm[:, sl],
                                     func=AF.Ln, bias=1.0)

            # odd halves of q/kv shifted to partitions 0-63 (one DMA)
            nc.sync.dma_start(out=lin0, in_=lin[D:128, :, :])

            # ---- phase C: per-chunk transposes + state deltas ----
            for c in range(NCH if stage >= 2 else 0):
                tok = slice(c * C, (c + 1) * C)
                tp = psp.tile([128, 3, C], BF16, tag="tT", name="tp")
                nc.tensor.matmul(tp[:, 0, :], lhsT=z_cm[:, tok], rhs=id128,
                                 start=True, stop=True, is_transpose=True)
                for g in range(2):
                    nc.tensor.matmul(tp[:, 1 + g, :], lhsT=lin[:, 2 + g, tok],
                                     rhs=id128, start=True, stop=True,
                                     is_transpose=True)
                nc.scalar.copy(z_sb[:, c, :], tp[:, 0, :])
                nc.scalar.copy(kvtok[:, c, :, :], tp[:, 1:3, :])

                dsp = psp.tile([128, 192], F32, tag="tS", name="dsp")
                for h in range(NH):
                    g, half = h // 2, h % 2
                    kvs = kvtok[:, c, g, 64 * half:64 * half + D]
                    zs = z_sb[:, c, h * PLEN:(h + 1) * PLEN]
                    # dS1[d, (h,p)]
                    nc.tensor.matmul(
                        dsp[0:D, 64 + h * PLEN:64 + (h + 1) * PLEN],
                        lhsT=kvs, rhs=zs, start=True, stop=True)
                    # dS2[(h,p), d] (col-packed)
                    nc.tensor.matmul(dsp[32 * h:32 * h + 32, 0:D],
                                     lhsT=zs, rhs=kvs, start=True, stop=True,
                                     tile_position=(0, 32 * h))
                nc.vector.tensor_copy(dS_sb[:, c, 0:D], dsp[:, 0:D])
                nc.vector.tensor_copy(dS_sb[0:D, c, 64:192], dsp[0:D, 64:192])

            # ---- phase D: prefix sums (serial DVE chain, small) ----
            for c in range(NCH if stage >= 2 else 0):
                cur, prv = c % 2, (c - 1) % 2
                if c == 0:
                    nc.vector.tensor_copy(Scum[:, 0, :], dS_sb[:, 0, :])
                else:
                    nc.vector.tensor_add(Scum[:, cur, :], dS_sb[:, c, :],
                                         Scum[:, prv, :])
                # S1b[c] : [d, (h,p)] bf16  (prefix THROUGH c)
                nc.vector.tensor_copy(S1b[:, c, :], Scum[0:D, cur, 64:192])
                # S2b[c] : block-diagonal [(h,p), h, d]
                for h in range(NH):
                    nc.vector.tensor_copy(
                        S2b[32 * h:32 * h + 32, c, h, :],
                        Scum[32 * h:32 * h + 32, cur, 0:D])

            # ---- phase E: per-chunk attention + output ----
            for c in range(NCH if stage >= 3 else 0):
                tok = slice(c * C, (c + 1) * C)
                # M1[key, query] per head
                m1 = psp.tile([128, NH, C], F32, tag="tA", name="m1")
                for h in range(NH):
                    nc.tensor.matmul(m1[:, h, :], lhsT=kv_at0(h, tok),
                                     rhs=q_at0(h, tok), start=True, stop=True)
                m1m = work.tile([128, NH, C], BF16, tag="m1m")
                nc.vector.tensor_mul(m1m, m1, _bcast(mask, NH))

                # out1[query, (h,p)] = intra + inter (token-major direct)
                o1 = psp.tile([128, NH, PLEN], F32, tag="tU", name="o1")
                for h in range(NH):
                    nc.tensor.matmul(o1[:, h, :],
                                     lhsT=m1m[:, h, :],
                                     rhs=z_sb[:, c, h * PLEN:(h + 1) * PLEN],
                                     start=True, stop=(c == 0))
                    if c > 0:
                        nc.tensor.matmul(
                            o1[:, h, :],
                            lhsT=q_at0(h, tok),
                            rhs=S1b[:, c - 1, h * PLEN:(h + 1) * PLEN],
                            start=False, stop=True)
                if stage < 4:
                    continue

                # softmax over plen (no max subtraction; |x| < 20 verified)
                e_sb = work.tile([128, NH, PLEN], F32, tag="e_sb")
                nc.scalar.activation(
                    out=e_sb, in_=o1,
                    func=AF.Exp, scale=rlen[:, c:c + 1])
                ssum = work.tile([128, NH], F32, tag="ssum")
                nc.vector.reduce_sum(ssum, e_sb, axis=AX.X)
                rs = work.tile([128, NH], F32, tag="rs")
                nc.vector.reciprocal(rs, ssum)
                rs2 = work.tile([128, NH], F32, tag="rs2")
                nc.vector.tensor_scalar_mul(rs2, rs, rlen[:, c:c + 1])
                aw = work.tile([128, NH, PLEN], BF16, tag="aw")
                nc.vector.tensor_mul(aw, e_sb, _bcast(rs2, PLEN, at=2))

                if stage < 5:
                    continue
                # awT[(h,p), query]
                awp = psp.tile([128, C], BF16, tag="tW", name="awp")
                nc.tensor.matmul(awp, lhsT=aw.rearrange("p h w -> p (h w)"),
                                 rhs=id128, start=True, stop=True,
                                 is_transpose=True)
                awT = work.tile([128, C], BF16, tag="awT")
                nc.scalar.copy(awT, awp)

                # M2[key, query] per head (rows 32h). Disjoint row groups run
                # CONCURRENTLY in the PE array, so consecutive heads must hit
                # different PSUM banks; alternate tags (tE/tT) so the tag
                # write-after-read dependency serializes same-bank reuse.
                m2m = work.tile([128, NH, C], BF16, tag="m2m")
                for h in range(NH):
                    p0 = 32 * h
                    m2h = psp.tile([128, C], F32,
                                   tag=("tE" if h % 2 == 0 else "tT"),
                                   name=f"m2h{h % 2}")
                    nc.tensor.matmul(m2h, lhsT=z_cm[p0:p0 + 32, tok],
                                     rhs=awT[p0:p0 + 32, :],
                                     start=True, stop=True,
                                     tile_position=(p0, 0))
                    nc.vector.tensor_mul(m2m[:, h, :], m2h, mask)

                if stage < 6:
                    continue
                # out2 = intra + inter, chained into one PSUM group per head
                attn = psp.tile([128, 2, C], F32, tag="tD", name="attn")
                for h in range(NH):
                    g, half = h // 2, h % 2
                    dst = attn[64 * half:64 * half + D, g, :]
                    nc.tensor.matmul(
                        dst,
                        lhsT=kvtok[:, c, g, 64 * half:64 * half + D],
                        rhs=m2m[:, h, :],
                        start=True, stop=(c == 0),
                        tile_position=(0, 64 * half))
                    if c > 0:
                        nc.tensor.matmul(dst, lhsT=S2b[:, c - 1, h, :],
                                         rhs=awT,
                                         start=False, stop=True,
                                         tile_position=(0, 64 * half))
                attnT = work.tile([128, 2, C], BF16, tag="attnT")
                nc.scalar.copy(attnT, attn)

                # final projection -> bf16 out (bo added on host)
                for nh in range(2):
                    osl = slice(nh * 512, (nh + 1) * 512)
                    fp = psp.tile([128, 512], F32, tag="tF", name="fp")
                    for kt in range(2):
                        nc.tensor.matmul(fp, lhsT=attnT[:, kt, :],
                                         rhs=wo[:, kt, osl],
                                         start=(kt == 0), stop=(kt == 1))
                    ob = outp.tile([128, 512], BF16, tag="ob")
                    nc.scalar.copy(ob, fp)
                    nc.sync.dma_start(out=out_d[tok, osl], in_=ob)

    nc.compile()
    return nc


_NC = None
_NC_STAGE = None


def get_nc(stage=6):
    global _NC, _NC_STAGE
    if _NC is None or _NC_STAGE != stage:
        _NC = build_nc(stage)
        _NC_STAGE = stage
    return _NC


def make_in_maps(query, pquery, Wpq, bpq, Wq, bq, Wpc, bpc, Wc, bc, Wo, bo):
    query = np.asarray(query, np.float32)
    pquery = np.asarray(pquery, np.float32)
    Wpq, Wq, Wpc, Wc, Wo = (np.asarray(w, np.float32)
                            for w in (Wpq, Wq, Wpc, Wc, Wo))
    bpq_, bq_, bpc_, bc_ = (np.asarray(v, np.float32)
                            for v in (bpq, bq, bpc, bc))
    n_idx = np.arange(NTOK, dtype=np.float64)
    rlen = (1.0 / ((n_idx + 1.0) * BETA)).astype(np.float32)
    rlen = np.ascontiguousarray(rlen.reshape(NCH, C).T)          # [C, NCH]
    mask = np.triu(np.ones((C, C), np.float32))                  # keep j <= i
    id128 = np.eye(128, dtype=np.float32)

    bf = ml_dtypes.bfloat16
    in_maps = []
    for core in range(8):
        b, hb = core // 4, core % 4
        ch = slice(hb * NH * D, (hb + 1) * NH * D)
        wqcT = np.concatenate([SCALING * Wq[ch], Wc[ch]], axis=0).T
        bqc = np.concatenate([SCALING * bq_[ch], bc_[ch]])       # (512,)
        bpqs = SCALING * bpq_[ch]                                # (256,)
        wpcR = np.ascontiguousarray(
            Wpc[ch].reshape(NH, D, 8, 128).transpose(1, 0, 2, 3))

        smf = np.zeros((128, SF_COLS), np.float32)
        smf[:, SF_BQC:SF_BQC + 4] = bqc.reshape(4, 128).T
        smf[0:D, SF_BPQ:SF_BPQ + NH] = bpqs.reshape(NH, D).T
        smf[:, SF_RLEN:SF_RLEN + NCH] = rlen
        smf[:, SF_MASK:SF_MASK + C] = mask

        smb = np.zeros((128, SB_COLS), np.float32)
        smb[:, SB_ID128:SB_ID128 + 128] = id128
        smb[0:D, SB_BPC:SB_BPC + NH] = bpc_[ch].reshape(NH, D).T

        in_maps.append({
            "xT": np.ascontiguousarray(query[:, b, :].T).astype(bf),
            "pxT": np.ascontiguousarray(pquery[:, b, :].T).astype(bf),
            "wqcT": np.ascontiguousarray(wqcT).astype(bf),
            "wpqT": np.ascontiguousarray((SCALING * Wpq[ch]).T).astype(bf),
            "wpcR": wpcR.astype(bf),
            "woT": np.ascontiguousarray(Wo[:, ch].T).astype(bf),
            "smf": smf,
            "smb": smb.astype(bf),
        })
    return in_maps


def kernel(**inputs):
    from concourse.bass_utils import run_bass_kernel_spmd
    nc = get_nc()
    in_maps = make_in_maps(**inputs)
    res = run_bass_kernel_spmd(nc, in_maps, core_ids=list(range(8)))
    bo = np.asarray(inputs["bo"], np.float32)
    out = np.zeros((NTOK, BSZ, EMBED), np.float32)
    for b in range(BSZ):
        acc = res.results[4 * b]["out"].astype(np.float32)
        for i in range(1, 4):
            acc = acc + res.results[4 * b + i]["out"].astype(np.float32)
        out[:, b, :] = acc + bo
    return out


# revision 10
# speedup vs baseline: 1.2238x; 1.0020x over previous
"""Self-contained Trainium2 Bass kernel for nn_LunarCausalAttention (v2).

Sharding: 8 cores = 2 batches x 4 head-blocks (4 heads each). Params sliced
per core host-side; per-core partial outputs (over head-blocks) summed on
host during the gather (plus bo). Output is bf16 on device, f32 on host.

v2 restructure vs v1: parallel-prefix chunk scan (phase C computes all
per-chunk state deltas independently; a small serial DVE prefix produces
per-chunk states; phase E consumes them chunk-independently), block-diagonal
padded S2 so inter-chunk terms chain into the intra PSUM accumulation group,
out1 computed transposed (cheap 32-col LDWEIGHTS) + one PE transpose,
single-DMA weight loads, bf16 output DMA.

PSUM tags (8 banks): tA lin/pattn/M1, tD lin/weff/attn, tE awp/M2-even,
tV M2-odd, tT phaseC transposes, tS beT/dS, tU pq/o1, tF proj.
"""

import math

import ml_dtypes
import numpy as np

import concourse.bacc as bacc
import concourse.bass as bass
import concourse.mybir as mybir
import concourse.tile as tile

EMBED = 1024
D = 64
PLEN = 32
NTOK = 2048
BSZ = 2
SCALING = D ** -0.5
BETA = math.log(2.0)

NH = 4           # heads per core
C = 128          # chunk (token tile)
NCH = NTOK // C  # 16 chunks
F32 = mybir.dt.float32
BF16 = mybir.dt.bfloat16
AX = mybir.AxisListType
AF = mybir.ActivationFunctionType

# smalls_f32 column layout
SF_BQC = 0          # [128, 4]
SF_BPQ = 4          # [64, 4] (head-major bpq at partitions 0-63)
SF_RLEN = 8         # [128, 16]
SF_MASK = 24        # [128, 128]
SF_COLS = 152
# smalls_bf16 column layout
SB_ID128 = 0        # [128, 128] bf16 identity
SB_BPC = 128        # [64, 4] bpc (heads cols)
SB_COLS = 132


def _bcast(ap_obj, dim_count, at=1):
    """Insert a stride-0 dim of size dim_count into an AP at free position."""
    pat = [list(p) for p in ap_obj.ap]
    pat.insert(at, [0, dim_count])
    return bass.AP(tensor=ap_obj.tensor, offset=ap_obj.offset, ap=pat)


def build_nc(stage=6):
    nc = bacc.Bacc("TRN2", target_bir_lowering=False, debug=False,
                   num_devices=8)

    xT_d = nc.dram_tensor("xT", [EMBED, NTOK], BF16, kind="ExternalInput")
    pxT_d = nc.dram_tensor("pxT", [EMBED, PLEN], BF16, kind="ExternalInput")
    wqc_d = nc.dram_tensor("wqcT", [EMBED, 4 * C], BF16, kind="ExternalInput")
    wpq_d = nc.dram_tensor("wpqT", [EMBED, 2 * C], BF16, kind="ExternalInput")
    wpc_d = nc.dram_tensor("wpcR", [D, NH, 8, 128], BF16, kind="ExternalInput")
    wo_d = nc.dram_tensor("woT", [NH * D, EMBED], BF16, kind="ExternalInput")
    sf_d = nc.dram_tensor("smf", [128, SF_COLS], F32, kind="ExternalInput")
    sb_d = nc.dram_tensor("smb", [128, SB_COLS], BF16, kind="ExternalInput")
    out_d = nc.dram_tensor("out", [NTOK, EMBED], BF16, kind="ExternalOutput")

    with tile.TileContext(nc) as tc:
        with (
            tc.tile_pool(name="big", bufs=1) as big,
            tc.tile_pool(name="work", bufs=2) as work,
            tc.tile_pool(name="outp", bufs=2) as outp,
            tc.tile_pool(name="psp", bufs=1, space="PSUM") as psp,
        ):
            # ---- persistent loads (each a single DMA) ----
            # order: xT q0 + pq-path weights first (critical path), rest after
            xT = big.tile([128, 8, NTOK], BF16)
            xT_r = xT_d.rearrange("(k p) n -> p k n", p=128)
            NQ = NTOK // 4
            nc.sync.dma_start(out=xT[:, :, 0:NQ], in_=xT_r[:, :, 0:NQ])
            wpq = big.tile([128, 8, 2 * C], BF16)
            nc.sync.dma_start(out=wpq,
                              in_=wpq_d.rearrange("(k p) m -> p k m", p=128))
            pxT = big.tile([128, 8, PLEN], BF16)
            nc.sync.dma_start(out=pxT,
                              in_=pxT_d.rearrange("(k p) n -> p k n", p=128))
            smf = big.tile([128, SF_COLS], F32)
            nc.sync.dma_start(out=smf, in_=sf_d.ap())
            smb = big.tile([128, SB_COLS], BF16)
            nc.sync.dma_start(out=smb, in_=sb_d.ap())
            wqc = big.tile([128, 8, 4 * C], BF16)
            nc.sync.dma_start(out=wqc,
                              in_=wqc_d.rearrange("(k p) m -> p k m", p=128))
            wpc = big.tile([D, NH, 8, 128], BF16)
            nc.sync.dma_start(out=wpc, in_=wpc_d.ap())
            wo = big.tile([128, 2, EMBED], BF16)
            nc.sync.dma_start(out=wo,
                              in_=wo_d.rearrange("(k p) o -> p k o", p=128))
            for qi in range(1, 4):
                nc.sync.dma_start(out=xT[:, :, qi * NQ:(qi + 1) * NQ],
                                  in_=xT_r[:, :, qi * NQ:(qi + 1) * NQ])

            bqc = smf[:, SF_BQC:SF_BQC + 4]
            bpq = smf[0:D, SF_BPQ:SF_BPQ + NH]
            rlen = smf[:, SF_RLEN:SF_RLEN + NCH]
            mask = smf[:, SF_MASK:SF_MASK + C]
            id128 = smb[:, SB_ID128:SB_ID128 + 128]
            bpc0 = smb[0:D, SB_BPC:SB_BPC + NH]

            # ---- persistent compute tensors ----
            lin = big.tile([128, 4, NTOK], BF16)      # q(0,1) kv(2,3) chan-major
            lin0 = big.tile([D, 4, NTOK], BF16)       # odd halves at base 0
            z_cm = big.tile([128, NTOK], BF16)        # [(h,p), tok]
            weff = big.tile([128, 8, NH * PLEN], BF16)
            pq0 = big.tile([D, NH, PLEN], BF16)
            beT = big.tile([128, 1], F32)             # beta * bias_eff per part
            zk = big.tile([128, NCH, 3, C], BF16)     # [tok, c, {z,kv0,kv1}]
            dS_sb = big.tile([128, NCH, 192], F32)    # [0:64]=dS2, [64:192]=dS1
            Scum = big.tile([128, 2, 192], F32)
            S1b = big.tile([D, NCH, NH * PLEN], BF16)  # prefix thru c
            S2b = big.tile([128, NCH, NH, D], BF16)    # block-diag padded

            nc.vector.memset(S2b, 0.0)
            nc.vector.memset(dS_sb[D:128, :, 64:192], 0.0)

            def q_at0(h, tok):
                g, half = h // 2, h % 2
                return (lin0[:, g, tok] if half else lin[0:D, g, tok])

            def kv_at0(h, tok):
                g, half = h // 2, h % 2
                return (lin0[:, 2 + g, tok] if half else lin[0:D, 2 + g, tok])

            # ---- pq linear, per head directly at partitions 0-63 ----
            pq_ps = psp.tile([D, NH, PLEN], F32, tag="tU", name="pq_ps")
            for h in range(NH):
                for k in range(8):
                    nc.tensor.matmul(pq_ps[:, h, :],
                                     lhsT=wpq[:, k, h * D:(h + 1) * D],
                                     rhs=pxT[:, k, :],
                                     start=(k == 0), stop=(k == 7))
            for h in range(NH):
                nc.scalar.activation(out=pq0[:, h, :], in_=pq_ps[:, h, :],
                                     func=AF.Identity, bias=bpq[:, h:h + 1],
                                     scale=1.0)

            # beT[(h,p)] = beta * (bpc_h . pq_h[:, p])  (col-packed matmuls)
            beT_ps = psp.tile([128, 512], F32, tag="tS", name="beT_ps")
            for h in range(NH):
                nc.tensor.matmul(beT_ps[32 * h:32 * h + 32, 0:1],
                                 lhsT=pq0[:, h, :], rhs=bpc0[:, h:h + 1],
                                 start=True, stop=True,
                                 tile_position=(0, 32 * h))
            nc.vector.tensor_scalar_mul(beT, beT_ps[:, 0:1], BETA)

            # ---- W_eff[e, (h,p)] = sum_d Wpc[(h,d), e] * pq[h, p, d] ----
            for k in range(8):
                ps = psp.tile([128, 512], F32, tag="tD", name="weff_ps")
                for h in range(NH):
                    nc.tensor.matmul(ps[:, h * PLEN:(h + 1) * PLEN],
                                     lhsT=wpc[:, h, k, :],
                                     rhs=pq0[:, h, :], start=True, stop=True)
                nc.scalar.copy(weff[:, k, :], ps[:, 0:NH * PLEN])

            # ---- q/kv linears + pattn + softplus, per token-quarter ----
            lin_tags = ("tA", "tD")
            for nt in range(4):
                sl = slice(nt * 512, (nt + 1) * 512)
                for m in range(4):
                    ps = psp.tile([128, 512], F32, tag=lin_tags[m % 2],
                                  name="lin_ps")
                    for k in range(8):
                        nc.tensor.matmul(ps,
                                         lhsT=wqc[:, k, m * 128:(m + 1) * 128],
                                         rhs=xT[:, k, sl],
                                         start=(k == 0), stop=(k == 7))
                    nc.scalar.activation(out=lin[:, m, sl], in_=ps,
                                         func=AF.Identity, bias=bqc[:, m:m + 1],
                                         scale=1.0)
                pps = psp.tile([128, 512], F32, tag="tA", name="pat_ps")
                for k in range(8):
                    nc.tensor.matmul(pps, lhsT=weff[:, k, :], rhs=xT[:, k, sl],
                                     start=(k == 0), stop=(k == 7))
                # z = ln(1 + exp(beta*pattn + beta*be)); /beta folded into rlen
                nc.scalar.activation(out=z_cm[:, sl], in_=pps, func=AF.Exp,
                                     bias=beT[:, 0:1], scale=BETA)
                nc.scalar.activation(out=z_cm[:, sl], in_=z_cm[:, sl],
                                     func=AF.Ln, bias=1.0)

            # odd halves of q/kv shifted to partitions 0-63 (one DMA)
            nc.sync.dma_start(out=lin0, in_=lin[D:128, :, :])

            # ---- phase C: per-chunk transposes + state deltas ----
            for c in range(NCH if stage >= 2 else 0):
                tok = slice(c * C, (c + 1) * C)
                tp = psp.tile([128, 3, C], BF16, tag="tT", name="tp")
                nc.tensor.matmul(tp[:, 0, :], lhsT=z_cm[:, tok], rhs=id128,
                                 start=True, stop=True, is_transpose=True)
                for g in range(2):
                    nc.tensor.matmul(tp[:, 1 + g, :], lhsT=lin[:, 2 + g, tok],
                                     rhs=id128, start=True, stop=True,
                                     is_transpose=True)
                nc.scalar.copy(zk[:, c, :, :], tp)

                dsp = psp.tile([128, 192], F32, tag="tS", name="dsp")
                for h in range(NH):
                    g, half = h // 2, h % 2
                    kvs = zk[:, c, 1 + g, 64 * half:64 * half + D]
                    zs = zk[:, c, 0, h * PLEN:(h + 1) * PLEN]
                    # dS1[d, (h,p)]
                    nc.tensor.matmul(
                        dsp[0:D, 64 + h * PLEN:64 + (h + 1) * PLEN],
                        lhsT=kvs, rhs=zs, start=True, stop=True)
                    # dS2[(h,p), d] (col-packed)
                    nc.tensor.matmul(dsp[32 * h:32 * h + 32, 0:D],
                                     lhsT=zs, rhs=kvs, start=True, stop=True,
                                     tile_position=(0, 32 * h))
                nc.vector.tensor_copy(dS_sb[:, c, 0:D], dsp[:, 0:D])
                nc.vector.tensor_copy(dS_sb[0:D, c, 64:192], dsp[0:D, 64:192])

            # ---- phase D: prefix sums (serial DVE chain, small) ----
            for c in range(NCH if stage >= 2 else 0):
                cur, prv = c % 2, (c - 1) % 2
                if c == 0:
                    nc.vector.tensor_copy(Scum[:, 0, :], dS_sb[:, 0, :])
                else:
                    nc.vector.tensor_add(Scum[:, cur, :], dS_sb[:, c, :],
                                         Scum[:, prv, :])
                # S1b[c] : [d, (h,p)] bf16  (prefix THROUGH c)
                nc.vector.tensor_copy(S1b[:, c, :], Scum[0:D, cur, 64:192])
                # S2b[c] : block-diagonal [(h,p), h, d]
                for h in range(NH):
                    nc.vector.tensor_copy(
                        S2b[32 * h:32 * h + 32, c, h, :],
                        Scum[32 * h:32 * h + 32, cur, 0:D])

            # ---- phase E: per-chunk attention + output ----
            for c in range(NCH if stage >= 3 else 0):
                tok = slice(c * C, (c + 1) * C)
                # M1[key, query] per head
                m1 = psp.tile([128, NH, C], F32, tag="tA", name="m1")
                for h in range(NH):
                    nc.tensor.matmul(m1[:, h, :], lhsT=kv_at0(h, tok),
                                     rhs=q_at0(h, tok), start=True, stop=True)
                m1m = work.tile([128, NH, C], BF16, tag="m1m")
                nc.vector.tensor_mul(m1m, m1, _bcast(mask, NH))

                # out1[query, (h,p)] = intra + inter (token-major direct)
                o1 = psp.tile([128, NH, PLEN], F32, tag="tU", name="o1")
                for h in range(NH):
                    nc.tensor.matmul(o1[:, h, :],
                                     lhsT=m1m[:, h, :],
                                     rhs=zk[:, c, 0, h * PLEN:(h + 1) * PLEN],
                                     start=True, stop=(c == 0))
                    if c > 0:
                        nc.tensor.matmul(
                            o1[:, h, :],
                            lhsT=q_at0(h, tok),
                            rhs=S1b[:, c - 1, h * PLEN:(h + 1) * PLEN],
                            start=False, stop=True)
                if stage < 4:
                    continue

                # softmax over plen (no max subtraction; |x| < 20 verified)
                e_sb = work.tile([128, NH, PLEN], F32, tag="e_sb")
                nc.scalar.activation(
                    out=e_sb, in_=o1,
                    func=AF.Exp, scale=rlen[:, c:c + 1])
                ssum = work.tile([128, NH], F32, tag="ssum")
                nc.vector.reduce_sum(ssum, e_sb, axis=AX.X)
                rs = work.tile([128, NH], F32, tag="rs")
                nc.vector.reciprocal(rs, ssum)
                rs2 = work.tile([128, NH], F32, tag="rs2")
                nc.vector.tensor_scalar_mul(rs2, rs, rlen[:, c:c + 1])
                aw = work.tile([128, NH, PLEN], BF16, tag="aw")
                nc.vector.tensor_mul(aw, e_sb, _bcast(rs2, PLEN, at=2))

                if stage < 5:
                    continue
                # awT[(h,p), query]
                awp = psp.tile([128, C], BF16, tag="tE", name="awp")
                nc.tensor.matmul(awp, lhsT=aw.rearrange("p h w -> p (h w)"),
                                 rhs=id128, start=True, stop=True,
                                 is_transpose=True)
                awT = work.tile([128, C], BF16, tag="awT")
                nc.scalar.copy(awT, awp)

                # M2[key, query] per head (rows 32h). Disjoint row groups run
                # CONCURRENTLY in the PE array, so consecutive heads must hit
                # different PSUM banks; alternate tags (tE/tT) so the tag
                # write-after-read dependency serializes same-bank reuse.
                m2m = work.tile([128, NH, C], BF16, tag="m2m")
                for h in range(NH):
                    p0 = 32 * h
                    m2h = psp.tile([128, C], F32,
                                   tag=("tE" if h % 2 == 0 else "tV"),
                                   name=f"m2h{h % 2}")
                    nc.tensor.matmul(m2h, lhsT=z_cm[p0:p0 + 32, tok],
                                     rhs=awT[p0:p0 + 32, :],
                                     start=True, stop=True,
                                     tile_position=(p0, 0))
                    nc.vector.tensor_mul(m2m[:, h, :], m2h, mask)

                if stage < 6:
                    continue
                # out2 = intra + inter, chained into one PSUM group per head
                attn = psp.tile([128, 2, C], F32, tag="tD", name="attn")
                for h in range(NH):
                    g, half = h // 2, h % 2
                    dst = attn[64 * half:64 * half + D, g, :]
                    nc.tensor.matmul(
                        dst,
                        lhsT=zk[:, c, 1 + g, 64 * half:64 * half + D],
                        rhs=m2m[:, h, :],
                        start=True, stop=(c == 0),
                        tile_position=(0, 64 * half))
                    if c > 0:
                        nc.tensor.matmul(dst, lhsT=S2b[:, c - 1, h, :],
                                         rhs=awT,
                                         start=False, stop=True,
                                         tile_position=(0, 64 * half))
                attnT = work.tile([128, 2, C], BF16, tag="attnT")
                nc.scalar.copy(attnT, attn)

                # final projection -> bf16 out (bo added on host)
                ob = outp.tile([128, EMBED], BF16, tag="ob")
                for nh in range(2):
                    osl = slice(nh * 512, (nh + 1) * 512)
                    fp = psp.tile([128, 512], F32, tag="tF", name="fp")
                    for kt in range(2):
                        nc.tensor.matmul(fp, lhsT=attnT[:, kt, :],
                                         rhs=wo[:, kt, osl],
                                         start=(kt == 0), stop=(kt == 1))
                    nc.scalar.copy(ob[:, osl], fp)
                nc.sync.dma_start(out=out_d[tok, :], in_=ob)

    nc.compile()
    return nc


_NC = None
_NC_STAGE = None


def get_nc(stage=6):
    global _NC, _NC_STAGE
    if _NC is None or _NC_STAGE != stage:
        _NC = build_nc(stage)
        _NC_STAGE = stage
    return _NC


def make_in_maps(query, pquery, Wpq, bpq, Wq, bq, Wpc, bpc, Wc, bc, Wo, bo):
    query = np.asarray(query, np.float32)
    pquery = np.asarray(pquery, np.float32)
    Wpq, Wq, Wpc, Wc, Wo = (np.asarray(w, np.float32)
                            for w in (Wpq, Wq, Wpc, Wc, Wo))
    bpq_, bq_, bpc_, bc_ = (np.asarray(v, np.float32)
                            for v in (bpq, bq, bpc, bc))
    n_idx = np.arange(NTOK, dtype=np.float64)
    rlen = (1.0 / ((n_idx + 1.0) * BETA)).astype(np.float32)
    rlen = np.ascontiguousarray(rlen.reshape(NCH, C).T)          # [C, NCH]
    mask = np.triu(np.ones((C, C), np.float32))                  # keep j <= i
    id128 = np.eye(128, dtype=np.float32)

    bf = ml_dtypes.bfloat16
    in_maps = []
    for core in range(8):
        b, hb = core // 4, core % 4
        ch = slice(hb * NH * D, (hb + 1) * NH * D)
        wqcT = np.concatenate([SCALING * Wq[ch], Wc[ch]], axis=0).T
        bqc = np.concatenate([SCALING * bq_[ch], bc_[ch]])       # (512,)
        bpqs = SCALING * bpq_[ch]                                # (256,)
        wpcR = np.ascontiguousarray(
            Wpc[ch].reshape(NH, D, 8, 128).transpose(1, 0, 2, 3))

        smf = np.zeros((128, SF_COLS), np.float32)
        smf[:, SF_BQC:SF_BQC + 4] = bqc.reshape(4, 128).T
        smf[0:D, SF_BPQ:SF_BPQ + NH] = bpqs.reshape(NH, D).T
        smf[:, SF_RLEN:SF_RLEN + NCH] = rlen
        smf[:, SF_MASK:SF_MASK + C] = mask

        smb = np.zeros((128, SB_COLS), np.float32)
        smb[:, SB_ID128:SB_ID128 + 128] = id128
        smb[0:D, SB_BPC:SB_BPC + NH] = bpc_[ch].reshape(NH, D).T

        in_maps.append({
            "xT": np.ascontiguousarray(query[:, b, :].T).astype(bf),
            "pxT": np.ascontiguousarray(pquery[:, b, :].T).astype(bf),
            "wqcT": np.ascontiguousarray(wqcT).astype(bf),
            "wpqT": np.ascontiguousarray((SCALING * Wpq[ch]).T).astype(bf),
            "wpcR": wpcR.astype(bf),
            "woT": np.ascontiguousarray(Wo[:, ch].T).astype(bf),
            "smf": smf,
            "smb": smb.astype(bf),
        })
    return in_maps


def kernel(**inputs):
    from concourse.bass_utils import run_bass_kernel_spmd
    nc = get_nc()
    in_maps = make_in_maps(**inputs)
    res = run_bass_kernel_spmd(nc, in_maps, core_ids=list(range(8)))
    bo = np.asarray(inputs["bo"], np.float32)
    out = np.zeros((NTOK, BSZ, EMBED), np.float32)
    for b in range(BSZ):
        acc = res.results[4 * b]["out"].astype(np.float32)
        for i in range(1, 4):
            acc = acc + res.results[4 * b + i]["out"].astype(np.float32)
        out[:, b, :] = acc + bo
    return out


# revision 17
# speedup vs baseline: 1.3381x; 1.0934x over previous
"""Self-contained Trainium2 Bass kernel for nn_LunarCausalAttention (v2).

Sharding: 8 cores = 2 batches x 4 head-blocks (4 heads each). Params sliced
per core host-side; per-core partial outputs (over head-blocks) summed on
host during the gather (plus bo). Output is bf16 on device, f32 on host.

v2 restructure vs v1: parallel-prefix chunk scan (phase C computes all
per-chunk state deltas independently; a small serial DVE prefix produces
per-chunk states; phase E consumes them chunk-independently), block-diagonal
padded S2 so inter-chunk terms chain into the intra PSUM accumulation group,
out1 computed transposed (cheap 32-col LDWEIGHTS) + one PE transpose,
single-DMA weight loads, bf16 output DMA.

PSUM tags (8 banks): tA lin/pattn/M1, tD lin/weff/attn, tE awp/M2-even,
tV M2-odd, tT phaseC transposes, tS beT/dS, tU pq/o1, tF proj.
"""

import math

import ml_dtypes
import numpy as np

import concourse.bacc as bacc
import concourse.bass as bass
import concourse.mybir as mybir
import concourse.tile as tile

EMBED = 1024
D = 64
PLEN = 32
NTOK = 2048
BSZ = 2
SCALING = D ** -0.5
BETA = math.log(2.0)

NH = 4           # heads per core
C = 128          # chunk (token tile)
NCH = NTOK // C  # 16 chunks
F32 = mybir.dt.float32
BF16 = mybir.dt.bfloat16
AX = mybir.AxisListType
AF = mybir.ActivationFunctionType

# smalls_f32 column layout
SF_BQC = 0          # [128, 4]
SF_BPQ = 4          # [64, 4] (head-major bpq at partitions 0-63)
SF_RLEN = 8         # [128, 16]
SF_MASK = 24        # [128, 128]
SF_COLS = 152
# smalls_bf16 column layout
SB_ID128 = 0        # [128, 128] bf16 identity
SB_BPC = 128        # [64, 4] bpc (heads cols)
SB_COLS = 132


def _bcast(ap_obj, dim_count, at=1):
    """Insert a stride-0 dim of size dim_count into an AP at free position."""
    pat = [list(p) for p in ap_obj.ap]
    pat.insert(at, [0, dim_count])
    return bass.AP(tensor=ap_obj.tensor, offset=ap_obj.offset, ap=pat)


def build_nc(stage=6):
    nc = bacc.Bacc("TRN2", target_bir_lowering=False, debug=False,
                   num_devices=8)

    xT_d = nc.dram_tensor("xT", [EMBED, NTOK], BF16, kind="ExternalInput")
    pxT_d = nc.dram_tensor("pxT", [EMBED, PLEN], BF16, kind="ExternalInput")
    wqc_d = nc.dram_tensor("wqcT", [EMBED, 4 * C], BF16, kind="ExternalInput")
    wpq_d = nc.dram_tensor("wpqT", [EMBED, 2 * C], BF16, kind="ExternalInput")
    wpc_d = nc.dram_tensor("wpcR", [D, NH, 8, 128], BF16, kind="ExternalInput")
    wo_d = nc.dram_tensor("woT", [NH * D, EMBED], BF16, kind="ExternalInput")
    sf_d = nc.dram_tensor("smf", [128, SF_COLS], F32, kind="ExternalInput")
    sb_d = nc.dram_tensor("smb", [128, SB_COLS], BF16, kind="ExternalInput")
    out_d = nc.dram_tensor("out", [NTOK, EMBED], BF16, kind="ExternalOutput")

    with tile.TileContext(nc) as tc:
        with (
            tc.tile_pool(name="big", bufs=1) as big,
            tc.tile_pool(name="work", bufs=2) as work,
            tc.tile_pool(name="outp", bufs=2) as outp,
            tc.tile_pool(name="psp", bufs=1, space="PSUM") as psp,
        ):
            # ---- persistent loads (each a single DMA) ----
            # order: xT q0 + pq-path weights first (critical path), rest after
            xT = big.tile([128, 8, NTOK], BF16)
            xT_r = xT_d.rearrange("(k p) n -> p k n", p=128)
            NQ = NTOK // 4
            nc.sync.dma_start(out=xT[:, :, 0:NQ], in_=xT_r[:, :, 0:NQ])
            wpq = big.tile([128, 8, 2 * C], BF16)
            nc.sync.dma_start(out=wpq,
                              in_=wpq_d.rearrange("(k p) m -> p k m", p=128))
            pxT = big.tile([128, 8, PLEN], BF16)
            nc.sync.dma_start(out=pxT,
                              in_=pxT_d.rearrange("(k p) n -> p k n", p=128))
            smf = big.tile([128, SF_COLS], F32)
            nc.sync.dma_start(out=smf, in_=sf_d.ap())
            smb = big.tile([128, SB_COLS], BF16)
            nc.sync.dma_start(out=smb, in_=sb_d.ap())
            wqc = big.tile([128, 8, 4 * C], BF16)
            nc.sync.dma_start(out=wqc,
                              in_=wqc_d.rearrange("(k p) m -> p k m", p=128))
            wpc = big.tile([D, NH, 8, 128], BF16)
            nc.sync.dma_start(out=wpc, in_=wpc_d.ap())
            wo = big.tile([128, 2, EMBED], BF16)
            nc.sync.dma_start(out=wo,
                              in_=wo_d.rearrange("(k p) o -> p k o", p=128))
            for qi in range(1, 4):
                nc.sync.dma_start(out=xT[:, :, qi * NQ:(qi + 1) * NQ],
                                  in_=xT_r[:, :, qi * NQ:(qi + 1) * NQ])

            bqc = smf[:, SF_BQC:SF_BQC + 4]
            bpq = smf[0:D, SF_BPQ:SF_BPQ + NH]
            rlen = smf[:, SF_RLEN:SF_RLEN + NCH]
            mask = smf[:, SF_MASK:SF_MASK + C]
            id128 = smb[:, SB_ID128:SB_ID128 + 128]
            bpc0 = smb[0:D, SB_BPC:SB_BPC + NH]

            # ---- persistent compute tensors ----
            lin = big.tile([128, 4, NTOK], BF16)      # q(0,1) kv(2,3) chan-major
            lin0 = big.tile([D, 4, NTOK], BF16)       # odd halves at base 0
            z_cm = big.tile([128, NTOK], BF16)        # [(h,p), tok]
            weff = big.tile([128, 8, NH * PLEN], BF16)
            pq0 = big.tile([D, NH, PLEN], BF16)
            beT = big.tile([128, 1], F32)             # beta * bias_eff per part
            zk = big.tile([128, NCH, 3, C], BF16)     # [tok, c, {z,kv0,kv1}]
            dS_sb = big.tile([128, NCH, 192], F32)    # [0:64]=dS2, [64:192]=dS1
            Scum = big.tile([128, 2, 192], F32)
            S1b = big.tile([D, NCH, NH * PLEN], BF16)  # prefix thru c
            S2b = big.tile([128, NCH, NH, D], BF16)    # block-diag padded

            nc.vector.memset(S2b, 0.0)
            nc.vector.memset(dS_sb[D:128, :, 64:192], 0.0)

            def q_at0(h, tok):
                g, half = h // 2, h % 2
                return (lin0[:, g, tok] if half else lin[0:D, g, tok])

            def kv_at0(h, tok):
                g, half = h // 2, h % 2
                return (lin0[:, 2 + g, tok] if half else lin[0:D, 2 + g, tok])

            # ---- pq linear, per head directly at partitions 0-63 ----
            pq_ps = psp.tile([D, NH, PLEN], F32, tag="tU", name="pq_ps")
            for h in range(NH):
                for k in range(8):
                    nc.tensor.matmul(pq_ps[:, h, :],
                                     lhsT=wpq[:, k, h * D:(h + 1) * D],
                                     rhs=pxT[:, k, :],
                                     start=(k == 0), stop=(k == 7))
            for h in range(NH):
                nc.scalar.activation(out=pq0[:, h, :], in_=pq_ps[:, h, :],
                                     func=AF.Identity, bias=bpq[:, h:h + 1],
                                     scale=1.0)

            # beT[(h,p)] = beta * (bpc_h . pq_h[:, p])  (col-packed matmuls)
            beT_ps = psp.tile([128, 512], F32, tag="tS", name="beT_ps")
            for h in range(NH):
                nc.tensor.matmul(beT_ps[32 * h:32 * h + 32, 0:1],
                                 lhsT=pq0[:, h, :], rhs=bpc0[:, h:h + 1],
                                 start=True, stop=True,
                                 tile_position=(0, 32 * h))
            nc.vector.tensor_scalar_mul(beT, beT_ps[:, 0:1], BETA)

            # ---- W_eff[e, (h,p)] = sum_d Wpc[(h,d), e] * pq[h, p, d] ----
            for k in range(8):
                ps = psp.tile([128, 512], F32, tag="tD", name="weff_ps")
                for h in range(NH):
                    nc.tensor.matmul(ps[:, h * PLEN:(h + 1) * PLEN],
                                     lhsT=wpc[:, h, k, :],
                                     rhs=pq0[:, h, :], start=True, stop=True)
                nc.scalar.copy(weff[:, k, :], ps[:, 0:NH * PLEN])

            # ---- q/kv linears + pattn + softplus, per token-quarter ----
            lin_tags = ("tA", "tD")
            for nt in range(4):
                sl = slice(nt * 512, (nt + 1) * 512)
                for m in range(4):
                    ps = psp.tile([128, 512], F32, tag=lin_tags[m % 2],
                                  name="lin_ps")
                    for k in range(8):
                        nc.tensor.matmul(ps,
                                         lhsT=wqc[:, k, m * 128:(m + 1) * 128],
                                         rhs=xT[:, k, sl],
                                         start=(k == 0), stop=(k == 7))
                    nc.scalar.activation(out=lin[:, m, sl], in_=ps,
                                         func=AF.Identity, bias=bqc[:, m:m + 1],
                                         scale=1.0)
                pps = psp.tile([128, 512], F32, tag="tA", name="pat_ps")
                for k in range(8):
                    nc.tensor.matmul(pps, lhsT=weff[:, k, :], rhs=xT[:, k, sl],
                                     start=(k == 0), stop=(k == 7))
                # z = ln(1 + exp(beta*pattn + beta*be)); /beta folded into rlen
                nc.scalar.activation(out=z_cm[:, sl], in_=pps, func=AF.Exp,
                                     bias=beT[:, 0:1], scale=BETA)
                nc.scalar.activation(out=z_cm[:, sl], in_=z_cm[:, sl],
                                     func=AF.Ln, bias=1.0)

                # odd halves of q/kv shifted to partitions 0-63, per quarter
                nc.sync.dma_start(out=lin0[:, :, sl], in_=lin[D:128, :, sl])

            # ---- phase C: per-chunk transposes + state deltas ----
            def phase_C(c):
                tok = slice(c * C, (c + 1) * C)
                tp = psp.tile([128, 3, C], BF16, tag="tT", name="tp")
                nc.tensor.matmul(tp[:, 0, :], lhsT=z_cm[:, tok], rhs=id128,
                                 start=True, stop=True, is_transpose=True)
                for g in range(2):
                    nc.tensor.matmul(tp[:, 1 + g, :], lhsT=lin[:, 2 + g, tok],
                                     rhs=id128, start=True, stop=True,
                                     is_transpose=True)
                nc.scalar.copy(zk[:, c, :, :], tp)

                dsp = psp.tile([128, 192], F32, tag="tS", name="dsp")
                for h in range(NH):
                    g, half = h // 2, h % 2
                    kvs = zk[:, c, 1 + g, 64 * half:64 * half + D]
                    zs = zk[:, c, 0, h * PLEN:(h + 1) * PLEN]
                    # dS1[d, (h,p)]
                    nc.tensor.matmul(
                        dsp[0:D, 64 + h * PLEN:64 + (h + 1) * PLEN],
                        lhsT=kvs, rhs=zs, start=True, stop=True)
                    # dS2[(h,p), d] (col-packed)
                    nc.tensor.matmul(dsp[32 * h:32 * h + 32, 0:D],
                                     lhsT=zs, rhs=kvs, start=True, stop=True,
                                     tile_position=(0, 32 * h))
                nc.vector.tensor_copy(dS_sb[:, c, 0:D], dsp[:, 0:D])
                nc.vector.tensor_copy(dS_sb[0:D, c, 64:192], dsp[0:D, 64:192])

            # ---- phase D: prefix sums (serial DVE chain, small) ----
            def phase_D(c):
                cur, prv = c % 2, (c - 1) % 2
                if c == 0:
                    nc.vector.tensor_copy(Scum[:, 0, :], dS_sb[:, 0, :])
                else:
                    nc.vector.tensor_add(Scum[:, cur, :], dS_sb[:, c, :],
                                         Scum[:, prv, :])
                # S1b[c] : [d, (h,p)] bf16  (prefix THROUGH c)
                nc.vector.tensor_copy(S1b[:, c, :], Scum[0:D, cur, 64:192])
                # S2b[c] : block-diagonal [(h,p), h, d]
                for h in range(NH):
                    nc.vector.tensor_copy(
                        S2b[32 * h:32 * h + 32, c, h, :],
                        Scum[32 * h:32 * h + 32, cur, 0:D])

            # ---- phase E: per-chunk attention + output ----
            def phase_E(c):
                tok = slice(c * C, (c + 1) * C)
                # M1[key, query] per head
                m1 = psp.tile([128, NH, C], F32, tag="tA", name="m1")
                for h in range(NH):
                    nc.tensor.matmul(m1[:, h, :], lhsT=kv_at0(h, tok),
                                     rhs=q_at0(h, tok), start=True, stop=True)
                m1m = work.tile([128, NH, C], BF16, tag="m1m")
                nc.vector.tensor_mul(m1m, m1, _bcast(mask, NH))

                # out1[query, (h,p)] = intra + inter (token-major direct)
                o1 = psp.tile([128, NH, PLEN], F32, tag="tU", name="o1")
                for h in range(NH):
                    nc.tensor.matmul(o1[:, h, :],
                                     lhsT=m1m[:, h, :],
                                     rhs=zk[:, c, 0, h * PLEN:(h + 1) * PLEN],
                                     start=True, stop=(c == 0))
                    if c > 0:
                        nc.tensor.matmul(
                            o1[:, h, :],
                            lhsT=q_at0(h, tok),
                            rhs=S1b[:, c - 1, h * PLEN:(h + 1) * PLEN],
                            start=False, stop=True)
                if stage < 4:
                    return

                # softmax over plen (no max subtraction; |x| < 20 verified)
                e_sb = work.tile([128, NH, PLEN], F32, tag="e_sb")
                nc.scalar.activation(
                    out=e_sb, in_=o1,
                    func=AF.Exp, scale=rlen[:, c:c + 1])
                ssum = work.tile([128, NH], F32, tag="ssum")
                nc.vector.reduce_sum(ssum, e_sb, axis=AX.X)
                rs = work.tile([128, NH], F32, tag="rs")
                nc.vector.reciprocal(rs, ssum)
                rs2 = work.tile([128, NH], F32, tag="rs2")
                nc.vector.tensor_scalar_mul(rs2, rs, rlen[:, c:c + 1])
                aw = work.tile([128, NH, PLEN], BF16, tag="aw")
                nc.vector.tensor_mul(aw, e_sb, _bcast(rs2, PLEN, at=2))

                if stage < 5:
                    return
                # awT[(h,p), query]
                awp = psp.tile([128, C], BF16, tag="tE", name="awp")
                nc.tensor.matmul(awp, lhsT=aw.rearrange("p h w -> p (h w)"),
                                 rhs=id128, start=True, stop=True,
                                 is_transpose=True)
                awT = work.tile([128, C], BF16, tag="awT")
                nc.scalar.copy(awT, awp)

                # M2[key, query] per head (rows 32h). Disjoint row groups run
                # CONCURRENTLY in the PE array, so consecutive heads must hit
                # different PSUM banks; alternate tags (tE/tT) so the tag
                # write-after-read dependency serializes same-bank reuse.
                m2m = work.tile([128, NH, C], BF16, tag="m2m")
                for h in range(NH):
                    p0 = 32 * h
                    m2h = psp.tile([128, C], F32,
                                   tag=("tE" if h % 2 == 0 else "tV"),
                                   name=f"m2h{h % 2}")
                    nc.tensor.matmul(m2h, lhsT=z_cm[p0:p0 + 32, tok],
                                     rhs=awT[p0:p0 + 32, :],
                                     start=True, stop=True,
                                     tile_position=(p0, 0))
                    nc.vector.tensor_mul(m2m[:, h, :], m2h, mask)

                if stage < 6:
                    return
                # out2 = intra + inter, chained into one PSUM group per head
                attn = psp.tile([128, 2, C], F32, tag="tD", name="attn")
                for h in range(NH):
                    g, half = h // 2, h % 2
                    dst = attn[64 * half:64 * half + D, g, :]
                    nc.tensor.matmul(
                        dst,
                        lhsT=zk[:, c, 1 + g, 64 * half:64 * half + D],
                        rhs=m2m[:, h, :],
                        start=True, stop=(c == 0),
                        tile_position=(0, 64 * half))
                    if c > 0:
                        nc.tensor.matmul(dst, lhsT=S2b[:, c - 1, h, :],
                                         rhs=awT,
                                         start=False, stop=True,
                                         tile_position=(0, 64 * half))
                attnT = work.tile([128, 2, C], BF16, tag="attnT")
                nc.scalar.copy(attnT, attn)

                # final projection -> bf16 out (bo added on host)
                ob = outp.tile([128, EMBED], BF16, tag="ob")
                for nh in range(2):
                    osl = slice(nh * 512, (nh + 1) * 512)
                    fp = psp.tile([128, 512], F32, tag="tF", name="fp")
                    for kt in range(2):
                        nc.tensor.matmul(fp, lhsT=attnT[:, kt, :],
                                         rhs=wo[:, kt, osl],
                                         start=(kt == 0), stop=(kt == 1))
                    nc.scalar.copy(ob[:, osl], fp)
                nc.sync.dma_start(out=out_d[tok, :], in_=ob)

            # ---- interleaved emission: C/D run ahead of E by LAG chunks so
            # independent phase-C matmuls fill phase-E's dependency gaps and
            # the PE stream stays dense (keeps the HAM clock-gate warm) ----
            LAG = 2
            if stage >= 2:
                for c in range(NCH):
                    phase_C(c)
                    phase_D(c)
                    if stage >= 3 and c >= LAG:
                        phase_E(c - LAG)
                if stage >= 3:
                    for c in range(NCH - LAG, NCH):
                        phase_E(c)

    nc.compile()
    return nc


_NC = None
_NC_STAGE = None


def get_nc(stage=6):
    global _NC, _NC_STAGE
    if _NC is None or _NC_STAGE != stage:
        _NC = build_nc(stage)
        _NC_STAGE = stage
    return _NC


def make_in_maps(query, pquery, Wpq, bpq, Wq, bq, Wpc, bpc, Wc, bc, Wo, bo):
    query = np.asarray(query, np.float32)
    pquery = np.asarray(pquery, np.float32)
    Wpq, Wq, Wpc, Wc, Wo = (np.asarray(w, np.float32)
                            for w in (Wpq, Wq, Wpc, Wc, Wo))
    bpq_, bq_, bpc_, bc_ = (np.asarray(v, np.float32)
                            for v in (bpq, bq, bpc, bc))
    n_idx = np.arange(NTOK, dtype=np.float64)
    rlen = (1.0 / ((n_idx + 1.0) * BETA)).astype(np.float32)
    rlen = np.ascontiguousarray(rlen.reshape(NCH, C).T)          # [C, NCH]
    mask = np.triu(np.ones((C, C), np.float32))                  # keep j <= i
    id128 = np.eye(128, dtype=np.float32)

    bf = ml_dtypes.bfloat16
    in_maps = []
    for core in range(8):
        b, hb = core // 4, core % 4
        ch = slice(hb * NH * D, (hb + 1) * NH * D)
        wqcT = np.concatenate([SCALING * Wq[ch], Wc[ch]], axis=0).T
        bqc = np.concatenate([SCALING * bq_[ch], bc_[ch]])       # (512,)
        bpqs = SCALING * bpq_[ch]                                # (256,)
        wpcR = np.ascontiguousarray(
            Wpc[ch].reshape(NH, D, 8, 128).transpose(1, 0, 2, 3))

        smf = np.zeros((128, SF_COLS), np.float32)
        smf[:, SF_BQC:SF_BQC + 4] = bqc.reshape(4, 128).T
        smf[0:D, SF_BPQ:SF_BPQ + NH] = bpqs.reshape(NH, D).T
        smf[:, SF_RLEN:SF_RLEN + NCH] = rlen
        smf[:, SF_MASK:SF_MASK + C] = mask

        smb = np.zeros((128, SB_COLS), np.float32)
        smb[:, SB_ID128:SB_ID128 + 128] = id128
        smb[0:D, SB_BPC:SB_BPC + NH] = bpc_[ch].reshape(NH, D).T

        in_maps.append({
            "xT": np.ascontiguousarray(query[:, b, :].T).astype(bf),
            "pxT": np.ascontiguousarray(pquery[:, b, :].T).astype(bf),
            "wqcT": np.ascontiguousarray(wqcT).astype(bf),
            "wpqT": np.ascontiguousarray((SCALING * Wpq[ch]).T).astype(bf),
            "wpcR": wpcR.astype(bf),
            "woT": np.ascontiguousarray(Wo[:, ch].T).astype(bf),
            "smf": smf,
            "smb": smb.astype(bf),
        })
    return in_maps


def kernel(**inputs):
    from concourse.bass_utils import run_bass_kernel_spmd
    nc = get_nc()
    in_maps = make_in_maps(**inputs)
    res = run_bass_kernel_spmd(nc, in_maps, core_ids=list(range(8)))
    bo = np.asarray(inputs["bo"], np.float32)
    out = np.zeros((NTOK, BSZ, EMBED), np.float32)
    for b in range(BSZ):
        acc = res.results[4 * b]["out"].astype(np.float32)
        for i in range(1, 4):
            acc = acc + res.results[4 * b + i]["out"].astype(np.float32)
        out[:, b, :] = acc + bo
    return out
